# revision 10
# baseline (speedup 1.0000x reference)
import sys

sys.path.insert(0, "/opt/trn_rl_repo")
import numpy as np

B, DIM, H, W = 2, 192, 64, 64
HEADS = 4
C = DIM // HEADS  # 48 per-head channels
HW = H * W  # 4096
NCORES = 8
KCH = HW // 128  # 32 k-chunks
NB = 8  # q blocks of 512
QW = HW // NB  # 512
UPT = 3  # S-chunk units per exp tile

_cache = {}


def _build():
    import concourse.bass as bass
    import concourse.tile as tile
    from concourse import bacc, mybir

    F32 = mybir.dt.float32
    BF16 = mybir.dt.float16
    AF = mybir.ActivationFunctionType

    nc = bacc.Bacc("TRN2", target_bir_lowering=False, debug=False,
                   num_devices=NCORES)
    x_d = nc.dram_tensor("x", [DIM, HW], BF16, kind="ExternalInput").ap()
    wqk_a_d = nc.dram_tensor("wqk_a", [128, 128], BF16, kind="ExternalInput").ap()
    wqk_b_d = nc.dram_tensor("wqk_b", [64, 128], BF16, kind="ExternalInput").ap()
    wv_a_d = nc.dram_tensor("wv_a", [128, C], BF16, kind="ExternalInput").ap()
    wv_b_d = nc.dram_tensor("wv_b", [64, C], BF16, kind="ExternalInput").ap()
    dwqk_d = nc.dram_tensor("dwqk", [128, 9 * 128], BF16, kind="ExternalInput").ap()
    dwv_d = nc.dram_tensor("dwv", [C, 9 * C], BF16, kind="ExternalInput").ap()
    pw_d = nc.dram_tensor("pw", [128, 128], BF16, kind="ExternalInput").ap()
    id_d = nc.dram_tensor("ident", [128, 128], F32, kind="ExternalInput").ap()
    tp_d = nc.dram_tensor("temp", [1, 1], F32, kind="ExternalInput").ap()
    out_d = nc.dram_tensor("out", [DIM, HW], F32, kind="ExternalOutput").ap()

    with tile.TileContext(nc) as tc:
        with (
            tc.tile_pool(name="persist", bufs=1) as pp,
            tc.tile_pool(name="epool", bufs=4) as ep,
        ):
            # ---- persistent SBUF tiles
            PK = pp.tile([128, HW], F32, tag="PK")     # q'@0:48, k'@64:112
            QHD = pp.tile([128, HW], BF16, tag="QHD")  # qhat bf16 dup 0:48/64:112
            KHb = pp.tile([128, HW], BF16, tag="KHb")  # temp*rk*k' dup'd
            vT = pp.tile([128, KCH * (C + 1)], BF16, tag="vT")  # v^T + ones col
            U = pp.tile([49, HW], F32, tag="U")        # av accum (row 48 = Z)
            ident = pp.tile([128, 128], F32, tag="ident")
            wqk_a = pp.tile([128, 128], BF16, tag="wqk_a")
            wqk_b = pp.tile([64, 128], BF16, tag="wqk_b")
            wv_a = pp.tile([128, C], BF16, tag="wv_a")
            wv_b = pp.tile([64, C], BF16, tag="wv_b")
            dwqk = pp.tile([128, 9 * 128], BF16, tag="dwqk")
            dwv = pp.tile([C, 9 * C], BF16, tag="dwv")
            pw = pp.tile([128, 128], BF16, tag="pw")
            temp_sb = pp.tile([1, 1], F32, tag="temp")
            ones_row = pp.tile([1, 128], F32, tag="ones_row")
            ones48 = pp.tile([128, 1], F32, tag="ones48")
            temp_col = pp.tile([128, 1], F32, tag="temp_col")
            negtemp_col = pp.tile([128, 1], F32, tag="negtemp_col")
            rr2 = pp.tile([128, 64], F32, tag="rr2")   # rq | rk (1/||.||)
            rkt = pp.tile([128, KCH], F32, tag="rkt")  # temp * rk, [p, chunk]
            att = pp.tile([128, HW], BF16, tag="att")
            z_row = pp.tile([1, HW], F32, tag="z_row")
            rz_row = pp.tile([1, HW], F32, tag="rz_row")
            rz = pp.tile([128, KCH], F32, tag="rz")

            nc.sync.dma_start(ident[:], id_d[:])
            nc.sync.dma_start(wqk_a[:], wqk_a_d[:])
            nc.sync.dma_start(wqk_b[:], wqk_b_d[:])
            nc.sync.dma_start(wv_a[:], wv_a_d[:])
            nc.sync.dma_start(wv_b[:], wv_b_d[:])
            nc.sync.dma_start(dwqk[:], dwqk_d[:])
            nc.sync.dma_start(dwv[:], dwv_d[:])
            nc.sync.dma_start(pw[:], pw_d[:])
            nc.sync.dma_start(temp_sb[:], tp_d[:])
            nc.gpsimd.memset(ones_row[:], 1.0)
            nc.gpsimd.memset(ones48[:], 1.0)
            nc.gpsimd.memset(vT[:], 1.0)

            with (
                tc.tile_pool(name="ph12", bufs=1) as p12,
                tc.tile_pool(name="psA", bufs=4, space="PSUM") as psA,
                tc.tile_pool(name="psB", bufs=2, space="PSUM") as psB,
            ):
                TQK = p12.tile([128, HW], BF16, tag="TQK")
                TV = p12.tile([C, HW], BF16, tag="TV")
                v_sb = p12.tile([C, HW], F32, tag="v_sb")

                with tc.tile_pool(name="xp", bufs=1) as xp:
                    x_a = xp.tile([128, HW], BF16, tag="x_a")
                    x_b = xp.tile([64, HW], BF16, tag="x_b")
                    for n in range(8):
                        s = slice(512 * n, 512 * (n + 1))
                        nc.sync.dma_start(x_a[:, s], x_d[0:128, s])
                        nc.sync.dma_start(x_b[:, s], x_d[128:192, s])

                    # temp_col = broadcast temp over partitions (K=1 matmul)
                    tP = psB.tile([128, 1], F32, tag="b")
                    nc.tensor.matmul(tP[:], ones_row[0:1, :], temp_sb[:],
                                     start=True, stop=True)
                    nc.scalar.copy(temp_col[:], tP[:])
                    nc.scalar.activation(negtemp_col[:], tP[:], AF.Copy,
                                         scale=-1.0)

                    # ---- phase 1: 1x1 conv; q out 0:48, k out 64:112 packed
                    for n in range(8):
                        s = slice(512 * n, 512 * (n + 1))
                        T1 = psA.tile([128, 512], F32, tag="a")
                        nc.tensor.matmul(T1[:], wqk_a[:], x_a[:, s],
                                         start=True, stop=False)
                        nc.tensor.matmul(T1[:], wqk_b[:], x_b[:, s],
                                         start=False, stop=True)
                        T1v = psB.tile([48, 512], F32, tag="b")
                        nc.tensor.matmul(T1v[:], wv_a[:], x_a[:, s],
                                         start=True, stop=False)
                        nc.tensor.matmul(T1v[:], wv_b[:], x_b[:, s],
                                         start=False, stop=True)
                        if n % 2 == 0:
                            nc.vector.tensor_copy(TQK[:, s], T1[:])
                            nc.scalar.copy(TV[:, s], T1v[:])
                        else:
                            nc.scalar.copy(TQK[:, s], T1[:])
                            nc.vector.tensor_copy(TV[:, s], T1v[:])

                ph35 = tc.tile_pool(name="ph35", bufs=1)
                p35 = ph35.__enter__()
                SQ = p35.tile([112, HW], F32, tag="SQ")
                ss_sb = p35.tile([33, HW], F32, tag="ss_sb")
                rq_row = p35.tile([1, HW], F32, tag="rq_row")
                rkt_row = p35.tile([1, HW], F32, tag="rkt_row")

                # ---- phase 2: depthwise 3x3 (block-diag qk lhsT, 9 taps)
                TQK3 = TQK[:].rearrange("p (y x) -> p y x", x=64)
                TV3 = TV[:].rearrange("p (y x) -> p y x", x=64)
                taps = [(0, 0)] + [(dy, dx) for dy in (-1, 0, 1)
                                   for dx in (-1, 0, 1) if (dy, dx) != (0, 0)]
                for n in range(8):
                    s = slice(512 * n, 512 * (n + 1))
                    DQK = psA.tile([128, 8, 64], F32, tag="a")
                    DV = psB.tile([48, 8, 64], F32, tag="b")
                    for ti, (dy, dx) in enumerate(taps):
                        t = (dy + 1) * 3 + (dx + 1)
                        first = ti == 0
                        last = ti == len(taps) - 1
                        gy0, gy1 = max(0, -dy), 64 - max(0, dy)
                        sy0, sy1 = max(8 * n, gy0), min(8 * n + 8, gy1)
                        if sy1 <= sy0:
                            continue
                        x0, x1 = max(0, -dx), 64 - max(0, dx)
                        oy = slice(sy0 - 8 * n, sy1 - 8 * n)
                        ox = slice(x0, x1)
                        iy = slice(sy0 + dy, sy1 + dy)
                        ix = slice(x0 + dx, x1 + dx)
                        nc.tensor.matmul(
                            DQK[:, oy, ox], dwqk[:, 128 * t:128 * t + 128],
                            TQK3[:, iy, ix], start=first, stop=last,
                            skip_group_check=True)
                        nc.tensor.matmul(
                            DV[:, oy, ox], dwv[:, C * t:C * t + 48],
                            TV3[:, iy, ix], start=first, stop=last,
                            skip_group_check=True)
                    DQKf = DQK[:].rearrange("p y x -> p (y x)")
                    DVf = DV[:].rearrange("p y x -> p (y x)")
                    if n % 2 == 0:
                        nc.vector.tensor_copy(PK[:, s], DQKf[:])
                        nc.scalar.copy(v_sb[:, s], DVf[:])
                    else:
                        nc.scalar.copy(PK[:, s], DQKf[:])
                        nc.vector.tensor_copy(v_sb[:, s], DVf[:])

                # ---- phase 3: squares + sum-of-squares (per position)
                for n in range(8):
                    s = slice(512 * n, 512 * (n + 1))
                    nc.vector.tensor_mul(SQ[:, s], PK[0:112, s], PK[0:112, s])
                    ssP = psA.tile([128, 512], F32, tag="a")
                    nc.tensor.matmul(ssP[0:1, :], ones48[0:48, :], SQ[0:48, s],
                                     start=True, stop=True)
                    nc.tensor.matmul(ssP[32:33, :], ones48[64:112, :],
                                     SQ[64:112, s], start=True, stop=True)
                    nc.vector.tensor_copy(ss_sb[:, s], ssP[0:33, :])

                # ---- phase 4: rsqrt via exp(-0.5*ln(ss)) in [128, 32] layout
                ssqT = psA.tile([128, 32], F32, tag="a")
                sskT = psA.tile([128, 32], F32, tag="a")
                for j in range(KCH):
                    cs = slice(128 * j, 128 * (j + 1))
                    nc.tensor.transpose(ssqT[:, j:j + 1], ss_sb[0:1, cs],
                                        ident[0:1, 0:1])
                    nc.tensor.transpose(sskT[:, j:j + 1], ss_sb[32:33, cs],
                                        ident[32:33, 32:33])
                lnb = p35.tile([128, 64], F32, tag="lnb")
                nc.scalar.activation(lnb[:, 0:32], ssqT[:], AF.Ln)
                nc.scalar.activation(lnb[:, 32:64], sskT[:], AF.Ln)
                nc.scalar.activation(rr2[:], lnb[:], AF.Exp, scale=-0.5)
                nc.scalar.activation(rkt[:], rr2[:, 32:64], AF.Copy,
                                     scale=temp_col[:])

                # ---- phase 5: rq/t*rk -> rows, broadcast, scale q and k
                for g in range(8):
                    rqP = psA.tile([1, 512], F32, tag="a")
                    rkP = psA.tile([1, 512], F32, tag="a")
                    for jj in range(4):
                        j = 4 * g + jj
                        nc.tensor.transpose(rqP[0:1, 128 * jj:128 * (jj + 1)],
                                            rr2[:, j:j + 1], ident[:])
                        nc.tensor.transpose(rkP[0:1, 128 * jj:128 * (jj + 1)],
                                            rkt[:, j:j + 1], ident[:])
                    nc.scalar.copy(rq_row[0:1, 512 * g:512 * (g + 1)], rqP[:])
                    nc.scalar.copy(rkt_row[0:1, 512 * g:512 * (g + 1)], rkP[:])
                for n in range(8):
                    s = slice(512 * n, 512 * (n + 1))
                    rqbP = psB.tile([48, 512], F32, tag="b")
                    nc.tensor.matmul(rqbP[:], ones_row[0:1, 0:48],
                                     rq_row[0:1, s], start=True, stop=True)
                    nc.vector.tensor_mul(QHD[0:48, s], PK[0:48, s], rqbP[:])
                    nc.sync.dma_start(QHD[64:112, s], QHD[0:48, s])
                    rkbP = psB.tile([48, 512], F32, tag="b")
                    nc.tensor.matmul(rkbP[:], ones_row[0:1, 0:48],
                                     rkt_row[0:1, s], start=True, stop=True)
                    nc.vector.tensor_mul(KHb[64:112, s], PK[64:112, s], rkbP[:])
                    nc.sync.dma_start(KHb[0:48, s], KHb[64:112, s])

                # ---- phase 6: v transpose -> vT chunks [128, 49] (ones col)
                for j in range(KCH):
                    vtP = psB.tile([128, 48], F32, tag="b")
                    nc.tensor.transpose(vtP[:], v_sb[:, 128 * j:128 * (j + 1)],
                                        ident[0:48, 0:48])
                    if j % 2 == 0:
                        nc.vector.tensor_copy(
                            vT[:, 49 * j:49 * j + 48], vtP[:])
                    else:
                        nc.scalar.copy(vT[:, 49 * j:49 * j + 48], vtP[:])
                ph35.__exit__(None, None, None)

            # ---- phase 7: attention, 8 q-blocks of 512
            with (
                tc.tile_pool(name="psS", bufs=2, space="PSUM") as psS,
                tc.tile_pool(name="psAV", bufs=1, space="PSUM") as psAV,
                tc.tile_pool(name="psZ", bufs=1, space="PSUM") as psZ,
            ):
                ntile = (KCH + UPT - 1) // UPT  # 11 tiles per block

                def zchain(g, step):
                    # deferred z-chain for block g, emitted piecewise
                    blk = slice(512 * g, 512 * (g + 1))
                    if step == 0:
                        nc.sync.dma_start(z_row[0:1, blk], U[48:49, blk])
                    elif step == 1:
                        zt = psZ.tile([128, 4], F32, tag="z")
                        for jj in range(4):
                            j = 4 * g + jj
                            nc.tensor.transpose(
                                zt[:, jj:jj + 1],
                                z_row[0:1, 128 * j:128 * (j + 1)],
                                ident[0:1, 0:1])
                        zchain.zt = zt
                    elif step == 2:
                        nc.vector.reciprocal(rz[:, 4 * g:4 * g + 4],
                                             zchain.zt[:])
                    elif step == 3:
                        rzP = psZ.tile([1, 512], F32, tag="z")
                        for jj in range(4):
                            j = 4 * g + jj
                            nc.tensor.transpose(
                                rzP[0:1, 128 * jj:128 * (jj + 1)],
                                rz[:, j:j + 1], ident[:])
                        zchain.rzP = rzP
                    elif step == 4:
                        nc.vector.tensor_copy(rz_row[0:1, blk], zchain.rzP[:])
                    elif step == 5:
                        rbP = psZ.tile([48, 512], F32, tag="z")
                        nc.tensor.matmul(rbP[:], ones_row[0:1, 0:48],
                                         rz_row[0:1, blk], start=True,
                                         stop=True)
                        zchain.rbP = rbP
                    elif step == 6:
                        nc.vector.tensor_mul(att[0:48, blk], U[0:48, blk],
                                             zchain.rbP[:])
                    elif step == 7:
                        nc.sync.dma_start(att[64:112, blk], att[0:48, blk])

                for g in range(NB):
                    blk = slice(512 * g, 512 * (g + 1))
                    avP = psAV.tile([49, 512], F32, tag="av")
                    pend = []
                    for j in range(ntile):
                        c0 = UPT * j
                        nu = min(UPT, KCH - c0)
                        T = psS.tile([128, UPT * 512], F32, tag="S")
                        for p in range(nu):
                            c = c0 + p
                            base = 0 if c % 2 == 0 else 64
                            nc.tensor.matmul(
                                T[:, 512 * p:512 * (p + 1)],
                                KHb[base:base + 48, 128 * c:128 * (c + 1)],
                                QHD[base:base + 48, blk],
                                start=True, stop=True)
                        E = ep.tile([128, UPT * 512], BF16, tag="E")
                        nc.scalar.activation(E[:, 0:512 * nu], T[:, 0:512 * nu],
                                             AF.Exp, bias=negtemp_col[:])
                        pend.append((c0, nu, E))
                        if j >= 2:
                            cc0, cnu, cE = pend.pop(0)
                            for p in range(cnu):
                                c = cc0 + p
                                nc.tensor.matmul(
                                    avP[:], vT[:, 49 * c:49 * c + 49],
                                    cE[:, 512 * p:512 * (p + 1)],
                                    start=(c == 0), stop=(c == KCH - 1),
                                    skip_group_check=True)
                        if g > 0 and 2 <= j <= 9:
                            zchain(g - 1, j - 2)
                    while pend:
                        cc0, cnu, cE = pend.pop(0)
                        for p in range(cnu):
                            c = cc0 + p
                            nc.tensor.matmul(
                                avP[:], vT[:, 49 * c:49 * c + 49],
                                cE[:, 512 * p:512 * (p + 1)],
                                start=(c == 0), stop=(c == KCH - 1),
                                skip_group_check=True)
                    nc.vector.tensor_copy(U[:, blk], avP[:])
                for step in range(8):
                    zchain(NB - 1, step)

            # ---- phase 8: proj, DMA out straight from PSUM
            with (
                tc.tile_pool(name="ph8", bufs=1) as p8,
                tc.tile_pool(name="psE", bufs=4, space="PSUM") as psE,
                tc.tile_pool(name="psF", bufs=2, space="PSUM") as psF,
            ):
                out_sb = p8.tile([128, HW], F32, tag="out_sb")
                out_sb2 = p8.tile([64, HW], F32, tag="out_sb2")
                for n in range(8):
                    s = slice(512 * n, 512 * (n + 1))
                    oP = psE.tile([128, 512], F32, tag="e")
                    oP2 = psF.tile([64, 512], F32, tag="f")
                    nc.tensor.matmul(oP[:], pw[0:48, 0:128], att[0:48, s],
                                     start=True, stop=True)
                    nc.tensor.matmul(oP2[:], pw[64:112, 0:64], att[64:112, s],
                                     start=True, stop=True)
                    if n % 2 == 0:
                        nc.vector.tensor_copy(out_sb[:, s], oP[:])
                        nc.scalar.copy(out_sb2[:, s], oP2[:])
                    else:
                        nc.scalar.copy(out_sb[:, s], oP[:])
                        nc.vector.tensor_copy(out_sb2[:, s], oP2[:])
                    nc.sync.dma_start(out_d[0:128, s], out_sb[:, s])
                    nc.sync.dma_start(out_d[128:192, s], out_sb2[:, s])

    nc.compile()
    return nc


def _get_nc():
    if "nc" not in _cache:
        _cache["nc"] = _build()
    return _cache["nc"]


def _prep_core(x, qkv_w, dw_w, proj_w, temperature, b, h):
    w1 = qkv_w[:, :, 0, 0]  # [576, 192]
    dw = dw_w[:, 0]  # [576, 3, 3]
    pwf = proj_w[:, :, 0, 0]  # [192, 192]
    qs, ks, vs = h * C, DIM + h * C, 2 * DIM + h * C
    wqk = np.zeros((DIM, 128), np.float32)  # lhsT: q cols 0:48, k cols 64:112
    wqk[:, 0:C] = w1[qs:qs + C].T
    wqk[:, 64:64 + C] = w1[ks:ks + C].T
    wv = np.ascontiguousarray(w1[vs:vs + C].T)  # [192, 48]
    dq, dk, dv = dw[qs:qs + C], dw[ks:ks + C], dw[vs:vs + C]
    dwqk = np.zeros((128, 9, 128), np.float32)
    dwv = np.zeros((C, 9, C), np.float32)
    ar = np.arange(C)
    for t in range(9):
        dy, dx = t // 3 - 1, t % 3 - 1
        dwqk[ar, t, ar] = dq[:, dy + 1, dx + 1]
        dwqk[64 + ar, t, 64 + ar] = dk[:, dy + 1, dx + 1]
        dwv[ar, t, ar] = dv[:, dy + 1, dx + 1]
    pw_sel = pwf[:, h * C:(h + 1) * C].T  # [48, 192]
    pwt = np.zeros((128, 128), np.float32)
    pwt[0:48, 0:128] = pw_sel[:, 0:128]
    pwt[64:112, 0:64] = pw_sel[:, 128:192]
    return {
        "x": np.ascontiguousarray(x[b].reshape(DIM, HW)).astype(np.float16),
        "wqk_a": wqk[0:128].astype(np.float16),
        "wqk_b": wqk[128:192].astype(np.float16),
        "wv_a": wv[0:128].astype(np.float16),
        "wv_b": wv[128:192].astype(np.float16),
        "dwqk": dwqk.reshape(128, 9 * 128).astype(np.float16),
        "dwv": dwv.reshape(C, 9 * C).astype(np.float16),
        "pw": pwt.astype(np.float16),
        "ident": np.eye(128, dtype=np.float32),
        "temp": np.array([[temperature[h, 0, 0]]], np.float32),
    }


def kernel(x, qkv_w, dw_w, proj_w, temperature):
    from concourse.bass_utils import run_bass_kernel_spmd

    nc = _get_nc()
    x = np.asarray(x, np.float32)
    qkv_w = np.asarray(qkv_w, np.float32)
    dw_w = np.asarray(dw_w, np.float32)
    proj_w = np.asarray(proj_w, np.float32)
    temperature = np.asarray(temperature, np.float32)
    in_maps = [
        _prep_core(x, qkv_w, dw_w, proj_w, temperature, c // HEADS, c % HEADS)
        for c in range(NCORES)
    ]
    res = run_bass_kernel_spmd(nc, in_maps, core_ids=list(range(NCORES)))
    out = np.zeros((B, DIM, HW), np.float32)
    for c in range(NCORES):
        out[c // HEADS] += res.results[c]["out"]
    return out.reshape(B, DIM, H, W)


# revision 17
# speedup vs baseline: 1.0619x; 1.0619x over previous
import sys

sys.path.insert(0, "/opt/trn_rl_repo")
import numpy as np

B, DIM, H, W = 2, 192, 64, 64
HEADS = 4
C = DIM // HEADS  # 48 per-head channels
HW = H * W  # 4096
NCORES = 8
KCH = HW // 128  # 32 k-chunks
NB = 8  # q blocks of 512
QW = HW // NB  # 512
UPT = 3  # S-chunk units per exp tile

_cache = {}


def _build():
    import concourse.bass as bass
    import concourse.tile as tile
    from concourse import bacc, mybir

    F32 = mybir.dt.float32
    BF16 = mybir.dt.float16
    AF = mybir.ActivationFunctionType

    nc = bacc.Bacc("TRN2", target_bir_lowering=False, debug=False,
                   num_devices=NCORES)
    x_d = nc.dram_tensor("x", [DIM, HW], BF16, kind="ExternalInput").ap()
    w1a_d = nc.dram_tensor("w1a", [128, 3 * C], BF16, kind="ExternalInput").ap()
    w1b_d = nc.dram_tensor("w1b", [64, 3 * C], BF16, kind="ExternalInput").ap()
    dwqk_d = nc.dram_tensor("dwqk", [128, 9 * C], BF16, kind="ExternalInput").ap()
    dwv_d = nc.dram_tensor("dwv", [C, 9 * C], BF16, kind="ExternalInput").ap()
    pw_d = nc.dram_tensor("pw", [128, 128], BF16, kind="ExternalInput").ap()
    id_d = nc.dram_tensor("ident", [128, 128], F32, kind="ExternalInput").ap()
    tp_d = nc.dram_tensor("temp", [1, 1], F32, kind="ExternalInput").ap()
    out_d = nc.dram_tensor("out", [DIM, HW], F32, kind="ExternalOutput").ap()

    with tile.TileContext(nc) as tc:
        with (
            tc.tile_pool(name="persist", bufs=1) as pp,
            tc.tile_pool(name="epool", bufs=4) as ep,
        ):
            # ---- persistent SBUF tiles
            PK = pp.tile([128, HW], F32, tag="PK")     # q'@0:48, k'@64:112
            QHD = pp.tile([128, HW], BF16, tag="QHD")  # qhat bf16 dup 0:48/64:112
            KHb = pp.tile([128, HW], BF16, tag="KHb")  # temp*rk*k' dup'd
            vT = pp.tile([128, KCH * (C + 1)], BF16, tag="vT")  # v^T + ones col
            U = pp.tile([49, HW], F32, tag="U")        # av accum (row 48 = Z)
            ident = pp.tile([128, 128], F32, tag="ident")
            w1a = pp.tile([128, 3 * C], BF16, tag="w1a")
            w1b = pp.tile([64, 3 * C], BF16, tag="w1b")
            dwqk = pp.tile([128, 9 * C], BF16, tag="dwqk")
            dwv = pp.tile([C, 9 * C], BF16, tag="dwv")
            pw = pp.tile([128, 128], BF16, tag="pw")
            temp_sb = pp.tile([1, 1], F32, tag="temp")
            ones_row = pp.tile([1, 128], F32, tag="ones_row")
            ones48 = pp.tile([128, 1], F32, tag="ones48")
            temp_col = pp.tile([128, 1], F32, tag="temp_col")
            negtemp_col = pp.tile([128, 1], F32, tag="negtemp_col")
            rr2 = pp.tile([128, 64], F32, tag="rr2")   # rq | rk (1/||.||)
            rkt = pp.tile([128, KCH], F32, tag="rkt")  # temp * rk, [p, chunk]
            att = pp.tile([128, HW], BF16, tag="att")
            z_row = pp.tile([1, HW], F32, tag="z_row")
            rz_row = pp.tile([1, HW], F32, tag="rz_row")
            rz = pp.tile([128, KCH], F32, tag="rz")

            nc.sync.dma_start(ident[:], id_d[:])
            nc.sync.dma_start(w1a[:], w1a_d[:])
            nc.sync.dma_start(w1b[:], w1b_d[:])
            nc.sync.dma_start(dwqk[:], dwqk_d[:])
            nc.sync.dma_start(dwv[:], dwv_d[:])
            nc.sync.dma_start(pw[:], pw_d[:])
            nc.sync.dma_start(temp_sb[:], tp_d[:])
            nc.gpsimd.memset(ones_row[:], 1.0)
            nc.gpsimd.memset(ones48[:], 1.0)
            nc.gpsimd.memset(vT[:], 1.0)

            with (
                tc.tile_pool(name="ph12", bufs=1) as p12,
                tc.tile_pool(name="psA", bufs=4, space="PSUM") as psA,
                tc.tile_pool(name="psB", bufs=2, space="PSUM") as psB,
            ):
                TQK = p12.tile([128, HW], BF16, tag="TQK")
                TV = p12.tile([C, HW], BF16, tag="TV")
                v_sb = p12.tile([C, HW], F32, tag="v_sb")

                with tc.tile_pool(name="xp", bufs=1) as xp:
                    x_a = xp.tile([128, HW], BF16, tag="x_a")
                    x_b = xp.tile([64, HW], BF16, tag="x_b")
                    for n in range(8):
                        s = slice(512 * n, 512 * (n + 1))
                        nc.sync.dma_start(x_a[:, s], x_d[0:128, s])
                        nc.sync.dma_start(x_b[:, s], x_d[128:192, s])

                    # temp_col = broadcast temp over partitions (K=1 matmul)
                    tP = psB.tile([128, 1], F32, tag="b")
                    nc.tensor.matmul(tP[:], ones_row[0:1, :], temp_sb[:],
                                     start=True, stop=True)
                    nc.scalar.copy(temp_col[:], tP[:])
                    nc.scalar.activation(negtemp_col[:], tP[:], AF.Copy,
                                         scale=-1.0)

                    # ---- phase 1: 1x1 conv (qkv_w), q/k col-packed, v separate
                    for n in range(8):
                        s = slice(512 * n, 512 * (n + 1))
                        T1 = psA.tile([128, 512], F32, tag="a")
                        nc.tensor.matmul(T1[0:48, :], w1a[:, 0:48], x_a[:, s],
                                         start=True, stop=False)
                        nc.tensor.matmul(T1[64:112, :], w1a[:, 48:96], x_a[:, s],
                                         start=True, stop=False)
                        nc.tensor.matmul(T1[0:48, :], w1b[:, 0:48], x_b[:, s],
                                         start=False, stop=True)
                        nc.tensor.matmul(T1[64:112, :], w1b[:, 48:96], x_b[:, s],
                                         start=False, stop=True)
                        T1v = psB.tile([48, 512], F32, tag="b")
                        nc.tensor.matmul(T1v[:], w1a[:, 96:144], x_a[:, s],
                                         start=True, stop=False)
                        nc.tensor.matmul(T1v[:], w1b[:, 96:144], x_b[:, s],
                                         start=False, stop=True)
                        if n % 2 == 0:
                            nc.vector.tensor_copy(TQK[:, s], T1[:])
                            nc.scalar.copy(TV[:, s], T1v[:])
                        else:
                            nc.scalar.copy(TQK[:, s], T1[:])
                            nc.vector.tensor_copy(TV[:, s], T1v[:])

                ph35 = tc.tile_pool(name="ph35", bufs=1)
                p35 = ph35.__enter__()
                SQ = p35.tile([112, HW], F32, tag="SQ")
                ss_sb = p35.tile([33, HW], F32, tag="ss_sb")
                rq_row = p35.tile([1, HW], F32, tag="rq_row")
                rkt_row = p35.tile([1, HW], F32, tag="rkt_row")

                # ---- phase 2: depthwise 3x3 (block-diag qk lhsT, 9 taps)
                TQK3 = TQK[:].rearrange("p (y x) -> p y x", x=64)
                TV3 = TV[:].rearrange("p (y x) -> p y x", x=64)
                taps = [(0, 0)] + [(dy, dx) for dy in (-1, 0, 1)
                                   for dx in (-1, 0, 1) if (dy, dx) != (0, 0)]
                for n in range(8):
                    s = slice(512 * n, 512 * (n + 1))
                    DQK = psA.tile([128, 8, 64], F32, tag="a")
                    DV = psB.tile([48, 8, 64], F32, tag="b")
                    for ti, (dy, dx) in enumerate(taps):
                        t = (dy + 1) * 3 + (dx + 1)
                        first = ti == 0
                        last = ti == len(taps) - 1
                        gy0, gy1 = max(0, -dy), 64 - max(0, dy)
                        sy0, sy1 = max(8 * n, gy0), min(8 * n + 8, gy1)
                        if sy1 <= sy0:
                            continue
                        x0, x1 = max(0, -dx), 64 - max(0, dx)
                        oy = slice(sy0 - 8 * n, sy1 - 8 * n)
                        ox = slice(x0, x1)
                        iy = slice(sy0 + dy, sy1 + dy)
                        ix = slice(x0 + dx, x1 + dx)
                        wsl = slice(C * t, C * t + 48)
                        nc.tensor.matmul(
                            DQK[0:48, oy, ox], dwqk[0:48, wsl],
                            TQK3[0:48, iy, ix], start=first, stop=last,
                            skip_group_check=True)
                        nc.tensor.matmul(
                            DQK[64:112, oy, ox], dwqk[64:112, wsl],
                            TQK3[64:112, iy, ix], start=first, stop=last,
                            skip_group_check=True)
                        nc.tensor.matmul(
                            DV[:, oy, ox], dwv[:, wsl],
                            TV3[:, iy, ix], start=first, stop=last,
                            skip_group_check=True)
                    DQKf = DQK[:].rearrange("p y x -> p (y x)")
                    DVf = DV[:].rearrange("p y x -> p (y x)")
                    if n % 2 == 0:
                        nc.vector.tensor_copy(PK[:, s], DQKf[:])
                        nc.scalar.copy(v_sb[:, s], DVf[:])
                    else:
                        nc.scalar.copy(PK[:, s], DQKf[:])
                        nc.vector.tensor_copy(v_sb[:, s], DVf[:])

                # ---- phase 3: squares + sum-of-squares (per position)
                for n in range(8):
                    s = slice(512 * n, 512 * (n + 1))
                    nc.vector.tensor_mul(SQ[:, s], PK[0:112, s], PK[0:112, s])
                    ssP = psA.tile([128, 512], F32, tag="a")
                    nc.tensor.matmul(ssP[0:1, :], ones48[0:48, :], SQ[0:48, s],
                                     start=True, stop=True)
                    nc.tensor.matmul(ssP[32:33, :], ones48[64:112, :],
                                     SQ[64:112, s], start=True, stop=True)
                    nc.vector.tensor_copy(ss_sb[:, s], ssP[0:33, :])

                # ---- phase 4: rsqrt via exp(-0.5*ln(ss)) in [128, 32] layout
                ssqT = psA.tile([128, 32], F32, tag="a")
                sskT = psA.tile([128, 32], F32, tag="a")
                for j in range(KCH):
                    cs = slice(128 * j, 128 * (j + 1))
                    nc.tensor.transpose(ssqT[:, j:j + 1], ss_sb[0:1, cs],
                                        ident[0:1, 0:1])
                    nc.tensor.transpose(sskT[:, j:j + 1], ss_sb[32:33, cs],
                                        ident[32:33, 32:33])
                lnb = p35.tile([128, 64], F32, tag="lnb")
                nc.scalar.activation(lnb[:, 0:32], ssqT[:], AF.Ln)
                nc.scalar.activation(lnb[:, 32:64], sskT[:], AF.Ln)
                nc.scalar.activation(rr2[:], lnb[:], AF.Exp, scale=-0.5)
                nc.scalar.activation(rkt[:], rr2[:, 32:64], AF.Copy,
                                     scale=temp_col[:])

                # ---- phase 5: rq/t*rk -> rows, broadcast, scale q and k
                for g in range(8):
                    rqP = psA.tile([1, 512], F32, tag="a")
                    rkP = psA.tile([1, 512], F32, tag="a")
                    for jj in range(4):
                        j = 4 * g + jj
                        nc.tensor.transpose(rqP[0:1, 128 * jj:128 * (jj + 1)],
                                            rr2[:, j:j + 1], ident[:])
                        nc.tensor.transpose(rkP[0:1, 128 * jj:128 * (jj + 1)],
                                            rkt[:, j:j + 1], ident[:])
                    nc.scalar.copy(rq_row[0:1, 512 * g:512 * (g + 1)], rqP[:])
                    nc.scalar.copy(rkt_row[0:1, 512 * g:512 * (g + 1)], rkP[:])
                for n in range(8):
                    s = slice(512 * n, 512 * (n + 1))
                    rqbP = psB.tile([48, 512], F32, tag="b")
                    nc.tensor.matmul(rqbP[:], ones_row[0:1, 0:48],
                                     rq_row[0:1, s], start=True, stop=True)
                    nc.vector.tensor_mul(QHD[0:48, s], PK[0:48, s], rqbP[:])
                    nc.sync.dma_start(QHD[64:112, s], QHD[0:48, s])
                    rkbP = psB.tile([48, 512], F32, tag="b")
                    nc.tensor.matmul(rkbP[:], ones_row[0:1, 0:48],
                                     rkt_row[0:1, s], start=True, stop=True)
                    nc.vector.tensor_mul(KHb[64:112, s], PK[64:112, s], rkbP[:])
                    nc.sync.dma_start(KHb[0:48, s], KHb[64:112, s])

                # ---- phase 6: v transpose -> vT chunks [128, 49] (ones col)
                for j in range(KCH):
                    vtP = psB.tile([128, 48], F32, tag="b")
                    nc.tensor.transpose(vtP[:], v_sb[:, 128 * j:128 * (j + 1)],
                                        ident[0:48, 0:48])
                    if j % 2 == 0:
                        nc.vector.tensor_copy(
                            vT[:, 49 * j:49 * j + 48], vtP[:])
                    else:
                        nc.scalar.copy(vT[:, 49 * j:49 * j + 48], vtP[:])
                ph35.__exit__(None, None, None)

            # ---- phase 7: attention, 8 q-blocks of 512
            with (
                tc.tile_pool(name="psS", bufs=2, space="PSUM") as psS,
                tc.tile_pool(name="psAV", bufs=1, space="PSUM") as psAV,
                tc.tile_pool(name="psZ", bufs=1, space="PSUM") as psZ,
            ):
                ntile = (KCH + UPT - 1) // UPT  # 11 tiles per block

                def zchain(g, step):
                    # deferred z-chain for block g, emitted piecewise
                    blk = slice(512 * g, 512 * (g + 1))
                    if step == 0:
                        nc.sync.dma_start(z_row[0:1, blk], U[48:49, blk])
                    elif step == 1:
                        zt = psZ.tile([128, 4], F32, tag="z")
                        for jj in range(4):
                            j = 4 * g + jj
                            nc.tensor.transpose(
                                zt[:, jj:jj + 1],
                                z_row[0:1, 128 * j:128 * (j + 1)],
                                ident[0:1, 0:1])
                        zchain.zt = zt
                    elif step == 2:
                        nc.vector.reciprocal(rz[:, 4 * g:4 * g + 4],
                                             zchain.zt[:])
                    elif step == 3:
                        rzP = psZ.tile([1, 512], F32, tag="z")
                        for jj in range(4):
                            j = 4 * g + jj
                            nc.tensor.transpose(
                                rzP[0:1, 128 * jj:128 * (jj + 1)],
                                rz[:, j:j + 1], ident[:])
                        zchain.rzP = rzP
                    elif step == 4:
                        nc.vector.tensor_copy(rz_row[0:1, blk], zchain.rzP[:])
                    elif step == 5:
                        rbP = psZ.tile([48, 512], F32, tag="z")
                        nc.tensor.matmul(rbP[:], ones_row[0:1, 0:48],
                                         rz_row[0:1, blk], start=True,
                                         stop=True)
                        zchain.rbP = rbP
                    elif step == 6:
                        nc.vector.tensor_mul(att[0:48, blk], U[0:48, blk],
                                             zchain.rbP[:])
                    elif step == 7:
                        nc.sync.dma_start(att[64:112, blk], att[0:48, blk])

                for g in range(NB):
                    blk = slice(512 * g, 512 * (g + 1))
                    avP = psAV.tile([49, 512], F32, tag="av")
                    pend = []
                    for j in range(ntile):
                        c0 = UPT * j
                        nu = min(UPT, KCH - c0)
                        T = psS.tile([128, UPT * 512], F32, tag="S")
                        for p in range(nu):
                            c = c0 + p
                            base = 0 if c % 2 == 0 else 64
                            nc.tensor.matmul(
                                T[:, 512 * p:512 * (p + 1)],
                                KHb[base:base + 48, 128 * c:128 * (c + 1)],
                                QHD[base:base + 48, blk],
                                start=True, stop=True)
                        E = ep.tile([128, UPT * 512], BF16, tag="E")
                        nc.scalar.activation(E[:, 0:512 * nu], T[:, 0:512 * nu],
                                             AF.Exp, bias=negtemp_col[:])
                        pend.append((c0, nu, E))
                        if j >= 2:
                            cc0, cnu, cE = pend.pop(0)
                            for p in range(cnu):
                                c = cc0 + p
                                nc.tensor.matmul(
                                    avP[:], vT[:, 49 * c:49 * c + 49],
                                    cE[:, 512 * p:512 * (p + 1)],
                                    start=(c == 0), stop=(c == KCH - 1),
                                    skip_group_check=True)
                        if g > 0 and 2 <= j <= 9:
                            zchain(g - 1, j - 2)
                    while pend:
                        cc0, cnu, cE = pend.pop(0)
                        for p in range(cnu):
                            c = cc0 + p
                            nc.tensor.matmul(
                                avP[:], vT[:, 49 * c:49 * c + 49],
                                cE[:, 512 * p:512 * (p + 1)],
                                start=(c == 0), stop=(c == KCH - 1),
                                skip_group_check=True)
                    nc.vector.tensor_copy(U[:, blk], avP[:])
                for step in range(8):
                    zchain(NB - 1, step)

            # ---- phase 8: proj, DMA out straight from PSUM
            with (
                tc.tile_pool(name="ph8", bufs=1) as p8,
                tc.tile_pool(name="psE", bufs=4, space="PSUM") as psE,
                tc.tile_pool(name="psF", bufs=2, space="PSUM") as psF,
            ):
                out_sb = p8.tile([128, HW], F32, tag="out_sb")
                out_sb2 = p8.tile([64, HW], F32, tag="out_sb2")
                for n in range(8):
                    s = slice(512 * n, 512 * (n + 1))
                    oP = psE.tile([128, 512], F32, tag="e")
                    oP2 = psF.tile([64, 512], F32, tag="f")
                    nc.tensor.matmul(oP[:], pw[0:48, 0:128], att[0:48, s],
                                     start=True, stop=True)
                    nc.tensor.matmul(oP2[:], pw[64:112, 0:64], att[64:112, s],
                                     start=True, stop=True)
                    if n % 2 == 0:
                        nc.vector.tensor_copy(out_sb[:, s], oP[:])
                        nc.scalar.copy(out_sb2[:, s], oP2[:])
                    else:
                        nc.scalar.copy(out_sb[:, s], oP[:])
                        nc.vector.tensor_copy(out_sb2[:, s], oP2[:])
                    nc.sync.dma_start(out_d[0:128, s], out_sb[:, s])
                    nc.sync.dma_start(out_d[128:192, s], out_sb2[:, s])

    nc.compile()
    return nc


def _get_nc():
    if "nc" not in _cache:
        _cache["nc"] = _build()
    return _cache["nc"]


def _prep_core(x, qkv_w, dw_w, proj_w, temperature, b, h):
    w1 = qkv_w[:, :, 0, 0]  # [576, 192]
    dw = dw_w[:, 0]  # [576, 3, 3]
    pwf = proj_w[:, :, 0, 0]  # [192, 192]
    qs, ks, vs = h * C, DIM + h * C, 2 * DIM + h * C
    sel = np.concatenate(
        [w1[qs:qs + C], w1[ks:ks + C], w1[vs:vs + C]], 0)  # [144, 192]
    lhsT = np.ascontiguousarray(sel.T)  # [192, 144]
    dq, dk, dv = dw[qs:qs + C], dw[ks:ks + C], dw[vs:vs + C]
    dwqk = np.zeros((128, 9, C), np.float32)
    dwv = np.zeros((C, 9, C), np.float32)
    ar = np.arange(C)
    for t in range(9):
        dy, dx = t // 3 - 1, t % 3 - 1
        dwqk[ar, t, ar] = dq[:, dy + 1, dx + 1]
        dwqk[64 + ar, t, ar] = dk[:, dy + 1, dx + 1]
        dwv[ar, t, ar] = dv[:, dy + 1, dx + 1]
    pw_sel = pwf[:, h * C:(h + 1) * C].T  # [48, 192]
    pwt = np.zeros((128, 128), np.float32)
    pwt[0:48, 0:128] = pw_sel[:, 0:128]
    pwt[64:112, 0:64] = pw_sel[:, 128:192]
    return {
        "x": np.ascontiguousarray(x[b].reshape(DIM, HW)).astype(np.float16),
        "w1a": lhsT[0:128].astype(np.float16),
        "w1b": lhsT[128:192].astype(np.float16),
        "dwqk": dwqk.reshape(128, 9 * C).astype(np.float16),
        "dwv": dwv.reshape(C, 9 * C).astype(np.float16),
        "pw": pwt.astype(np.float16),
        "ident": np.eye(128, dtype=np.float32),
        "temp": np.array([[temperature[h, 0, 0]]], np.float32),
    }


def kernel(x, qkv_w, dw_w, proj_w, temperature):
    from concourse.bass_utils import run_bass_kernel_spmd

    nc = _get_nc()
    x = np.asarray(x, np.float32)
    qkv_w = np.asarray(qkv_w, np.float32)
    dw_w = np.asarray(dw_w, np.float32)
    proj_w = np.asarray(proj_w, np.float32)
    temperature = np.asarray(temperature, np.float32)
    in_maps = [
        _prep_core(x, qkv_w, dw_w, proj_w, temperature, c // HEADS, c % HEADS)
        for c in range(NCORES)
    ]
    res = run_bass_kernel_spmd(nc, in_maps, core_ids=list(range(NCORES)))
    out = np.zeros((B, DIM, HW), np.float32)
    for c in range(NCORES):
        out[c // HEADS] += res.results[c]["out"]
    return out.reshape(B, DIM, H, W)


# revision 18
# speedup vs baseline: 1.1692x; 1.1011x over previous
import sys

sys.path.insert(0, "/opt/trn_rl_repo")
import numpy as np

B, DIM, H, W = 2, 192, 64, 64
HEADS = 4
C = DIM // HEADS  # 48 per-head channels
HW = H * W  # 4096
NCORES = 8
KCH = HW // 128  # 32 k-chunks
NB = 8  # q blocks of 512
QW = HW // NB  # 512
UPT = 3  # S-chunk units per exp tile

_cache = {}


def _build():
    import concourse.bass as bass
    import concourse.tile as tile
    from concourse import bacc, mybir

    F32 = mybir.dt.float32
    BF16 = mybir.dt.float16
    AF = mybir.ActivationFunctionType

    nc = bacc.Bacc("TRN2", target_bir_lowering=False, debug=False,
                   num_devices=NCORES)
    x_d = nc.dram_tensor("x", [DIM, HW], BF16, kind="ExternalInput").ap()
    w1a_d = nc.dram_tensor("w1a", [128, 3 * C], BF16, kind="ExternalInput").ap()
    w1b_d = nc.dram_tensor("w1b", [64, 3 * C], BF16, kind="ExternalInput").ap()
    dwqk_d = nc.dram_tensor("dwqk", [128, 9 * C], BF16, kind="ExternalInput").ap()
    dwv_d = nc.dram_tensor("dwv", [C, 9 * C], BF16, kind="ExternalInput").ap()
    pw_d = nc.dram_tensor("pw", [128, 128], BF16, kind="ExternalInput").ap()
    id_d = nc.dram_tensor("ident", [128, 128], F32, kind="ExternalInput").ap()
    tp_d = nc.dram_tensor("temp", [1, 1], F32, kind="ExternalInput").ap()
    out_d = nc.dram_tensor("out", [DIM, HW], F32, kind="ExternalOutput").ap()

    with tile.TileContext(nc) as tc:
        with (
            tc.tile_pool(name="persist", bufs=1) as pp,
            tc.tile_pool(name="epool", bufs=4) as ep,
        ):
            # ---- persistent SBUF tiles
            PK = pp.tile([128, HW], F32, tag="PK")     # q'@0:48, k'@64:112
            QHD = pp.tile([128, HW], BF16, tag="QHD")  # qhat bf16 dup 0:48/64:112
            KHb = pp.tile([128, HW], BF16, tag="KHb")  # temp*rk*k' dup'd
            vT = pp.tile([128, KCH * (C + 1)], BF16, tag="vT")  # v^T + ones col
            U = pp.tile([49, HW], F32, tag="U")        # av accum (row 48 = Z)
            ident = pp.tile([128, 128], F32, tag="ident")
            w1a = pp.tile([128, 3 * C], BF16, tag="w1a")
            w1b = pp.tile([64, 3 * C], BF16, tag="w1b")
            dwqk = pp.tile([128, 9 * C], BF16, tag="dwqk")
            dwv = pp.tile([C, 9 * C], BF16, tag="dwv")
            pw = pp.tile([128, 128], BF16, tag="pw")
            temp_sb = pp.tile([1, 1], F32, tag="temp")
            ones_row = pp.tile([1, 128], F32, tag="ones_row")
            ones48 = pp.tile([128, 1], F32, tag="ones48")
            temp_col = pp.tile([128, 1], F32, tag="temp_col")
            negtemp_col = pp.tile([128, 1], F32, tag="negtemp_col")
            rr2 = pp.tile([128, 64], F32, tag="rr2")   # rq | rk (1/||.||)
            rkt = pp.tile([128, KCH], F32, tag="rkt")  # temp * rk, [p, chunk]
            att = pp.tile([128, HW], BF16, tag="att")
            z_row = pp.tile([1, HW], F32, tag="z_row")
            rz_row = pp.tile([1, HW], F32, tag="rz_row")
            rz = pp.tile([128, KCH], F32, tag="rz")

            nc.sync.dma_start(ident[:], id_d[:])
            nc.sync.dma_start(w1a[:], w1a_d[:])
            nc.sync.dma_start(w1b[:], w1b_d[:])
            nc.sync.dma_start(dwqk[:], dwqk_d[:])
            nc.sync.dma_start(dwv[:], dwv_d[:])
            nc.sync.dma_start(pw[:], pw_d[:])
            nc.sync.dma_start(temp_sb[:], tp_d[:])
            nc.gpsimd.memset(ones_row[:], 1.0)
            nc.gpsimd.memset(ones48[:], 1.0)
            nc.gpsimd.memset(vT[:], 1.0)

            with (
                tc.tile_pool(name="ph12", bufs=1) as p12,
                tc.tile_pool(name="psA", bufs=4, space="PSUM") as psA,
                tc.tile_pool(name="psB", bufs=2, space="PSUM") as psB,
            ):
                TQK = p12.tile([128, HW], BF16, tag="TQK")
                TV = p12.tile([C, HW], BF16, tag="TV")
                v_sb = p12.tile([C, HW], F32, tag="v_sb")

                with tc.tile_pool(name="xp", bufs=1) as xp:
                    x_a = xp.tile([128, HW], BF16, tag="x_a")
                    x_b = xp.tile([64, HW], BF16, tag="x_b")
                    nc.sync.dma_start(x_a[:], x_d[0:128, :])
                    nc.sync.dma_start(x_b[:], x_d[128:192, :])

                    # temp_col = broadcast temp over partitions (K=1 matmul)
                    tP = psB.tile([128, 1], F32, tag="b")
                    nc.tensor.matmul(tP[:], ones_row[0:1, :], temp_sb[:],
                                     start=True, stop=True)
                    nc.scalar.copy(temp_col[:], tP[:])
                    nc.scalar.activation(negtemp_col[:], tP[:], AF.Copy,
                                         scale=-1.0)

                    # ---- phase 1: 1x1 conv (qkv_w), q/k col-packed, v separate
                    for n in range(8):
                        s = slice(512 * n, 512 * (n + 1))
                        T1 = psA.tile([128, 512], F32, tag="a")
                        nc.tensor.matmul(T1[0:48, :], w1a[:, 0:48], x_a[:, s],
                                         start=True, stop=False)
                        nc.tensor.matmul(T1[64:112, :], w1a[:, 48:96], x_a[:, s],
                                         start=True, stop=False)
                        nc.tensor.matmul(T1[0:48, :], w1b[:, 0:48], x_b[:, s],
                                         start=False, stop=True)
                        nc.tensor.matmul(T1[64:112, :], w1b[:, 48:96], x_b[:, s],
                                         start=False, stop=True)
                        T1v = psB.tile([48, 512], F32, tag="b")
                        nc.tensor.matmul(T1v[:], w1a[:, 96:144], x_a[:, s],
                                         start=True, stop=False)
                        nc.tensor.matmul(T1v[:], w1b[:, 96:144], x_b[:, s],
                                         start=False, stop=True)
                        if n % 2 == 0:
                            nc.vector.tensor_copy(TQK[:, s], T1[:])
                            nc.scalar.copy(TV[:, s], T1v[:])
                        else:
                            nc.scalar.copy(TQK[:, s], T1[:])
                            nc.vector.tensor_copy(TV[:, s], T1v[:])

                ph35 = tc.tile_pool(name="ph35", bufs=1)
                p35 = ph35.__enter__()
                SQ = p35.tile([112, HW], F32, tag="SQ")
                ss_sb = p35.tile([33, HW], F32, tag="ss_sb")
                rq_row = p35.tile([1, HW], F32, tag="rq_row")
                rkt_row = p35.tile([1, HW], F32, tag="rkt_row")

                # ---- phase 2: depthwise 3x3 (block-diag qk lhsT, 9 taps)
                TQK3 = TQK[:].rearrange("p (y x) -> p y x", x=64)
                TV3 = TV[:].rearrange("p (y x) -> p y x", x=64)
                taps = [(0, 0)] + [(dy, dx) for dy in (-1, 0, 1)
                                   for dx in (-1, 0, 1) if (dy, dx) != (0, 0)]
                for n in range(8):
                    s = slice(512 * n, 512 * (n + 1))
                    DQK = psA.tile([128, 8, 64], F32, tag="a")
                    DV = psB.tile([48, 8, 64], F32, tag="b")
                    for ti, (dy, dx) in enumerate(taps):
                        t = (dy + 1) * 3 + (dx + 1)
                        first = ti == 0
                        last = ti == len(taps) - 1
                        gy0, gy1 = max(0, -dy), 64 - max(0, dy)
                        sy0, sy1 = max(8 * n, gy0), min(8 * n + 8, gy1)
                        if sy1 <= sy0:
                            continue
                        x0, x1 = max(0, -dx), 64 - max(0, dx)
                        oy = slice(sy0 - 8 * n, sy1 - 8 * n)
                        ox = slice(x0, x1)
                        iy = slice(sy0 + dy, sy1 + dy)
                        ix = slice(x0 + dx, x1 + dx)
                        wsl = slice(C * t, C * t + 48)
                        nc.tensor.matmul(
                            DQK[0:48, oy, ox], dwqk[0:48, wsl],
                            TQK3[0:48, iy, ix], start=first, stop=last,
                            skip_group_check=True)
                        nc.tensor.matmul(
                            DQK[64:112, oy, ox], dwqk[64:112, wsl],
                            TQK3[64:112, iy, ix], start=first, stop=last,
                            skip_group_check=True)
                        nc.tensor.matmul(
                            DV[:, oy, ox], dwv[:, wsl],
                            TV3[:, iy, ix], start=first, stop=last,
                            skip_group_check=True)
                    DQKf = DQK[:].rearrange("p y x -> p (y x)")
                    DVf = DV[:].rearrange("p y x -> p (y x)")
                    if n % 2 == 0:
                        nc.vector.tensor_copy(PK[:, s], DQKf[:])
                        nc.scalar.copy(v_sb[:, s], DVf[:])
                    else:
                        nc.scalar.copy(PK[:, s], DQKf[:])
                        nc.vector.tensor_copy(v_sb[:, s], DVf[:])

                # ---- phase 3: squares + sum-of-squares (per position)
                for n in range(8):
                    s = slice(512 * n, 512 * (n + 1))
                    nc.vector.tensor_mul(SQ[:, s], PK[0:112, s], PK[0:112, s])
                    ssP = psA.tile([128, 512], F32, tag="a")
                    nc.tensor.matmul(ssP[0:1, :], ones48[0:48, :], SQ[0:48, s],
                                     start=True, stop=True)
                    nc.tensor.matmul(ssP[32:33, :], ones48[64:112, :],
                                     SQ[64:112, s], start=True, stop=True)
                    nc.vector.tensor_copy(ss_sb[:, s], ssP[0:33, :])

                # ---- phase 4: rsqrt via exp(-0.5*ln(ss)) in [128, 32] layout
                ssqT = psA.tile([128, 32], F32, tag="a")
                sskT = psA.tile([128, 32], F32, tag="a")
                for j in range(KCH):
                    cs = slice(128 * j, 128 * (j + 1))
                    nc.tensor.transpose(ssqT[:, j:j + 1], ss_sb[0:1, cs],
                                        ident[0:1, 0:1])
                    nc.tensor.transpose(sskT[:, j:j + 1], ss_sb[32:33, cs],
                                        ident[32:33, 32:33])
                lnb = p35.tile([128, 64], F32, tag="lnb")
                nc.scalar.activation(lnb[:, 0:32], ssqT[:], AF.Ln)
                nc.scalar.activation(lnb[:, 32:64], sskT[:], AF.Ln)
                nc.scalar.activation(rr2[:], lnb[:], AF.Exp, scale=-0.5)
                nc.scalar.activation(rkt[:], rr2[:, 32:64], AF.Copy,
                                     scale=temp_col[:])

                # ---- phase 5: rq/t*rk -> rows, broadcast, scale q and k
                for g in range(8):
                    rqP = psA.tile([1, 512], F32, tag="a")
                    rkP = psA.tile([1, 512], F32, tag="a")
                    for jj in range(4):
                        j = 4 * g + jj
                        nc.tensor.transpose(rqP[0:1, 128 * jj:128 * (jj + 1)],
                                            rr2[:, j:j + 1], ident[:])
                        nc.tensor.transpose(rkP[0:1, 128 * jj:128 * (jj + 1)],
                                            rkt[:, j:j + 1], ident[:])
                    nc.scalar.copy(rq_row[0:1, 512 * g:512 * (g + 1)], rqP[:])
                    nc.scalar.copy(rkt_row[0:1, 512 * g:512 * (g + 1)], rkP[:])
                for n in range(8):
                    s = slice(512 * n, 512 * (n + 1))
                    rqbP = psB.tile([48, 512], F32, tag="b")
                    nc.tensor.matmul(rqbP[:], ones_row[0:1, 0:48],
                                     rq_row[0:1, s], start=True, stop=True)
                    nc.vector.tensor_mul(QHD[0:48, s], PK[0:48, s], rqbP[:])
                    nc.sync.dma_start(QHD[64:112, s], QHD[0:48, s])
                    rkbP = psB.tile([48, 512], F32, tag="b")
                    nc.tensor.matmul(rkbP[:], ones_row[0:1, 0:48],
                                     rkt_row[0:1, s], start=True, stop=True)
                    nc.vector.tensor_mul(KHb[64:112, s], PK[64:112, s], rkbP[:])
                    nc.sync.dma_start(KHb[0:48, s], KHb[64:112, s])

                # ---- phase 6: v transpose -> vT chunks [128, 49] (ones col)
                for j in range(KCH):
                    vtP = psB.tile([128, 48], F32, tag="b")
                    nc.tensor.transpose(vtP[:], v_sb[:, 128 * j:128 * (j + 1)],
                                        ident[0:48, 0:48])
                    if j % 2 == 0:
                        nc.vector.tensor_copy(
                            vT[:, 49 * j:49 * j + 48], vtP[:])
                    else:
                        nc.scalar.copy(vT[:, 49 * j:49 * j + 48], vtP[:])
                ph35.__exit__(None, None, None)

            # ---- phase 7: attention, 8 q-blocks of 512
            with (
                tc.tile_pool(name="psS", bufs=2, space="PSUM") as psS,
                tc.tile_pool(name="psAV", bufs=1, space="PSUM") as psAV,
                tc.tile_pool(name="psZ", bufs=1, space="PSUM") as psZ,
            ):
                ntile = (KCH + UPT - 1) // UPT  # 11 tiles per block

                def zchain(g, step):
                    # deferred z-chain for block g, emitted piecewise
                    blk = slice(512 * g, 512 * (g + 1))
                    if step == 0:
                        nc.sync.dma_start(z_row[0:1, blk], U[48:49, blk])
                    elif step == 1:
                        zt = psZ.tile([128, 4], F32, tag="z")
                        for jj in range(4):
                            j = 4 * g + jj
                            nc.tensor.transpose(
                                zt[:, jj:jj + 1],
                                z_row[0:1, 128 * j:128 * (j + 1)],
                                ident[0:1, 0:1])
                        zchain.zt = zt
                    elif step == 2:
                        nc.vector.reciprocal(rz[:, 4 * g:4 * g + 4],
                                             zchain.zt[:])
                    elif step == 3:
                        rzP = psZ.tile([1, 512], F32, tag="z")
                        for jj in range(4):
                            j = 4 * g + jj
                            nc.tensor.transpose(
                                rzP[0:1, 128 * jj:128 * (jj + 1)],
                                rz[:, j:j + 1], ident[:])
                        zchain.rzP = rzP
                    elif step == 4:
                        nc.vector.tensor_copy(rz_row[0:1, blk], zchain.rzP[:])
                    elif step == 5:
                        rbP = psZ.tile([48, 512], F32, tag="z")
                        nc.tensor.matmul(rbP[:], ones_row[0:1, 0:48],
                                         rz_row[0:1, blk], start=True,
                                         stop=True)
                        zchain.rbP = rbP
                    elif step == 6:
                        nc.vector.tensor_mul(att[0:48, blk], U[0:48, blk],
                                             zchain.rbP[:])
                    elif step == 7:
                        nc.sync.dma_start(att[64:112, blk], att[0:48, blk])

                for g in range(NB):
                    blk = slice(512 * g, 512 * (g + 1))
                    avP = psAV.tile([49, 512], F32, tag="av")
                    pend = []
                    for j in range(ntile):
                        c0 = UPT * j
                        nu = min(UPT, KCH - c0)
                        T = psS.tile([128, UPT * 512], F32, tag="S")
                        for p in range(nu):
                            c = c0 + p
                            base = 0 if c % 2 == 0 else 64
                            nc.tensor.matmul(
                                T[:, 512 * p:512 * (p + 1)],
                                KHb[base:base + 48, 128 * c:128 * (c + 1)],
                                QHD[base:base + 48, blk],
                                start=True, stop=True)
                        E = ep.tile([128, UPT * 512], BF16, tag="E")
                        nc.scalar.activation(E[:, 0:512 * nu], T[:, 0:512 * nu],
                                             AF.Exp, bias=negtemp_col[:])
                        pend.append((c0, nu, E))
                        if j >= 2:
                            cc0, cnu, cE = pend.pop(0)
                            for p in range(cnu):
                                c = cc0 + p
                                nc.tensor.matmul(
                                    avP[:], vT[:, 49 * c:49 * c + 49],
                                    cE[:, 512 * p:512 * (p + 1)],
                                    start=(c == 0), stop=(c == KCH - 1),
                                    skip_group_check=True)
                        if g > 0 and 2 <= j <= 9:
                            zchain(g - 1, j - 2)
                    while pend:
                        cc0, cnu, cE = pend.pop(0)
                        for p in range(cnu):
                            c = cc0 + p
                            nc.tensor.matmul(
                                avP[:], vT[:, 49 * c:49 * c + 49],
                                cE[:, 512 * p:512 * (p + 1)],
                                start=(c == 0), stop=(c == KCH - 1),
                                skip_group_check=True)
                    nc.vector.tensor_copy(U[:, blk], avP[:])
                for step in range(8):
                    zchain(NB - 1, step)

            # ---- phase 8: proj, DMA out straight from PSUM
            with (
                tc.tile_pool(name="ph8", bufs=1) as p8,
                tc.tile_pool(name="psE", bufs=4, space="PSUM") as psE,
                tc.tile_pool(name="psF", bufs=2, space="PSUM") as psF,
            ):
                out_sb = p8.tile([128, HW], F32, tag="out_sb")
                out_sb2 = p8.tile([64, HW], F32, tag="out_sb2")
                for n in range(8):
                    s = slice(512 * n, 512 * (n + 1))
                    oP = psE.tile([128, 512], F32, tag="e")
                    oP2 = psF.tile([64, 512], F32, tag="f")
                    nc.tensor.matmul(oP[:], pw[0:48, 0:128], att[0:48, s],
                                     start=True, stop=True)
                    nc.tensor.matmul(oP2[:], pw[64:112, 0:64], att[64:112, s],
                                     start=True, stop=True)
                    if n % 2 == 0:
                        nc.vector.tensor_copy(out_sb[:, s], oP[:])
                        nc.scalar.copy(out_sb2[:, s], oP2[:])
                    else:
                        nc.scalar.copy(out_sb[:, s], oP[:])
                        nc.vector.tensor_copy(out_sb2[:, s], oP2[:])
                    nc.sync.dma_start(out_d[0:128, s], out_sb[:, s])
                    nc.sync.dma_start(out_d[128:192, s], out_sb2[:, s])

    nc.compile()
    return nc


def _get_nc():
    if "nc" not in _cache:
        _cache["nc"] = _build()
    return _cache["nc"]


def _prep_core(x, qkv_w, dw_w, proj_w, temperature, b, h):
    w1 = qkv_w[:, :, 0, 0]  # [576, 192]
    dw = dw_w[:, 0]  # [576, 3, 3]
    pwf = proj_w[:, :, 0, 0]  # [192, 192]
    qs, ks, vs = h * C, DIM + h * C, 2 * DIM + h * C
    sel = np.concatenate(
        [w1[qs:qs + C], w1[ks:ks + C], w1[vs:vs + C]], 0)  # [144, 192]
    lhsT = np.ascontiguousarray(sel.T)  # [192, 144]
    dq, dk, dv = dw[qs:qs + C], dw[ks:ks + C], dw[vs:vs + C]
    dwqk = np.zeros((128, 9, C), np.float32)
    dwv = np.zeros((C, 9, C), np.float32)
    ar = np.arange(C)
    for t in range(9):
        dy, dx = t // 3 - 1, t % 3 - 1
        dwqk[ar, t, ar] = dq[:, dy + 1, dx + 1]
        dwqk[64 + ar, t, ar] = dk[:, dy + 1, dx + 1]
        dwv[ar, t, ar] = dv[:, dy + 1, dx + 1]
    pw_sel = pwf[:, h * C:(h + 1) * C].T  # [48, 192]
    pwt = np.zeros((128, 128), np.float32)
    pwt[0:48, 0:128] = pw_sel[:, 0:128]
    pwt[64:112, 0:64] = pw_sel[:, 128:192]
    return {
        "x": np.ascontiguousarray(x[b].reshape(DIM, HW)).astype(np.float16),
        "w1a": lhsT[0:128].astype(np.float16),
        "w1b": lhsT[128:192].astype(np.float16),
        "dwqk": dwqk.reshape(128, 9 * C).astype(np.float16),
        "dwv": dwv.reshape(C, 9 * C).astype(np.float16),
        "pw": pwt.astype(np.float16),
        "ident": np.eye(128, dtype=np.float32),
        "temp": np.array([[temperature[h, 0, 0]]], np.float32),
    }


def kernel(x, qkv_w, dw_w, proj_w, temperature):
    from concourse.bass_utils import run_bass_kernel_spmd

    nc = _get_nc()
    x = np.asarray(x, np.float32)
    qkv_w = np.asarray(qkv_w, np.float32)
    dw_w = np.asarray(dw_w, np.float32)
    proj_w = np.asarray(proj_w, np.float32)
    temperature = np.asarray(temperature, np.float32)
    in_maps = [
        _prep_core(x, qkv_w, dw_w, proj_w, temperature, c // HEADS, c % HEADS)
        for c in range(NCORES)
    ]
    res = run_bass_kernel_spmd(nc, in_maps, core_ids=list(range(NCORES)))
    out = np.zeros((B, DIM, HW), np.float32)
    for c in range(NCORES):
        out[c // HEADS] += res.results[c]["out"]
    return out.reshape(B, DIM, H, W)


# revision 37
# speedup vs baseline: 1.1851x; 1.0136x over previous
import sys

sys.path.insert(0, "/opt/trn_rl_repo")
import numpy as np

B, DIM, H, W = 2, 192, 64, 64
HEADS = 4
C = DIM // HEADS  # 48 per-head channels
HW = H * W  # 4096
NCORES = 8
KCH = HW // 128  # 32 k-chunks
NB = 8  # q blocks of 512
QW = HW // NB  # 512
UPT = 3  # S-chunk units per exp tile

_cache = {}


def _build(debug_taps=False):
    import concourse.bass as bass
    import concourse.tile as tile
    from concourse import bacc, mybir

    F32 = mybir.dt.float32
    BF16 = mybir.dt.float16
    AF = mybir.ActivationFunctionType

    nc = bacc.Bacc("TRN2", target_bir_lowering=False, debug=False,
                   num_devices=NCORES)
    x_d = nc.dram_tensor("x", [DIM, HW], BF16, kind="ExternalInput").ap()
    w1a_d = nc.dram_tensor("w1a", [128, 3 * C], BF16, kind="ExternalInput").ap()
    w1b_d = nc.dram_tensor("w1b", [64, 3 * C], BF16, kind="ExternalInput").ap()
    dwqk_d = nc.dram_tensor("dwqk", [128, 9 * C], BF16, kind="ExternalInput").ap()
    dwv_d = nc.dram_tensor("dwv", [C, 9 * C], BF16, kind="ExternalInput").ap()
    pw_d = nc.dram_tensor("pw", [128, 128], BF16, kind="ExternalInput").ap()
    id_d = nc.dram_tensor("ident", [128, 128], F32, kind="ExternalInput").ap()
    tp_d = nc.dram_tensor("temp", [1, 1], F32, kind="ExternalInput").ap()
    out_d = nc.dram_tensor("out", [DIM, HW], F32, kind="ExternalOutput").ap()
    if debug_taps:
        dbg_khb = nc.dram_tensor("dbg_khb", [128, HW], BF16,
                                 kind="ExternalOutput").ap()
        dbg_qhd = nc.dram_tensor("dbg_qhd", [128, HW], BF16,
                                 kind="ExternalOutput").ap()
        dbg_vt = nc.dram_tensor("dbg_vt", [128, KCH * 64], BF16,
                                kind="ExternalOutput").ap()
        dbg_u = nc.dram_tensor("dbg_u", [49, HW], F32,
                               kind="ExternalOutput").ap()
        dbg_att = nc.dram_tensor("dbg_att", [128, HW], BF16,
                                 kind="ExternalOutput").ap()

    with tile.TileContext(nc) as tc:
        with (
            tc.tile_pool(name="persist", bufs=1) as pp,
            tc.tile_pool(name="epool", bufs=4) as ep,
        ):
            # ---- persistent SBUF tiles
            PK = pp.tile([128, HW], F32, tag="PK")     # q'@0:48, k'@64:112
            QHD = pp.tile([128, HW], BF16, tag="QHD")  # qhat bf16 dup 0:48/64:112
            KHb = pp.tile([128, HW], BF16, tag="KHb")  # temp*rk*k' dup'd
            # v^T chunks at 64-col stride (XBAR dest needs 128B alignment);
            # col 64c+48 is the ones column for the Z row
            vT = pp.tile([128, KCH * 64], BF16, tag="vT")
            v_sb = pp.tile([C, HW], BF16, tag="v_sb")
            U = pp.tile([49, HW], F32, tag="U")        # av accum (row 48 = Z)
            rq_row = pp.tile([1, HW], F32, tag="rq_row")
            rkt_row = pp.tile([1, HW], F32, tag="rkt_row")
            ident = pp.tile([128, 128], F32, tag="ident")
            w1a = pp.tile([128, 3 * C], BF16, tag="w1a")
            w1b = pp.tile([64, 3 * C], BF16, tag="w1b")
            dwqk = pp.tile([128, 9 * C], BF16, tag="dwqk")
            dwv = pp.tile([C, 9 * C], BF16, tag="dwv")
            pw = pp.tile([128, 128], BF16, tag="pw")
            temp_sb = pp.tile([1, 1], F32, tag="temp")
            ones_row = pp.tile([1, 128], F32, tag="ones_row")
            ones48 = pp.tile([128, 1], F32, tag="ones48")
            temp_col = pp.tile([128, 1], F32, tag="temp_col")
            negtemp_col = pp.tile([128, 1], F32, tag="negtemp_col")
            rr2 = pp.tile([128, 64], F32, tag="rr2")   # rq | rk (1/||.||)
            rkt = pp.tile([128, KCH], F32, tag="rkt")  # temp * rk, [p, chunk]
            att = pp.tile([128, HW], BF16, tag="att")
            rz_row = pp.tile([1, HW], F32, tag="rz_row")
            rz = pp.tile([128, KCH], F32, tag="rz")

            nc.sync.dma_start(ident[:], id_d[:])
            nc.sync.dma_start(w1a[:], w1a_d[:])
            nc.sync.dma_start(w1b[:], w1b_d[:])
            nc.sync.dma_start(dwqk[:], dwqk_d[:])
            nc.sync.dma_start(dwv[:], dwv_d[:])
            nc.sync.dma_start(pw[:], pw_d[:])
            nc.sync.dma_start(temp_sb[:], tp_d[:])
            nc.gpsimd.memset(ones_row[:], 1.0)
            nc.gpsimd.memset(ones48[:], 1.0)
            nc.gpsimd.memset(vT[:], 1.0)
            # preload the Ln ACT table set during the x-DMA wait
            dummy = pp.tile([1, 1], F32, tag="dummy")
            nc.scalar.activation(dummy[:], ones_row[0:1, 0:1], AF.Ln)

            with (
                tc.tile_pool(name="ph12", bufs=1) as p12,
                tc.tile_pool(name="psA", bufs=4, space="PSUM") as psA,
                tc.tile_pool(name="psB", bufs=2, space="PSUM") as psB,
            ):
                TQK = p12.tile([128, HW], BF16, tag="TQK")
                TV = p12.tile([C, HW], BF16, tag="TV")

                with tc.tile_pool(name="xp", bufs=1) as xp:
                    x_a = xp.tile([128, HW], BF16, tag="x_a")
                    x_b = xp.tile([64, HW], BF16, tag="x_b")
                    nc.sync.dma_start(x_a[:], x_d[0:128, :])
                    nc.sync.dma_start(x_b[:], x_d[128:192, :])

                    # temp_col = broadcast temp over partitions (K=1 matmul)
                    tP = psB.tile([128, 1], F32, tag="b")
                    nc.tensor.matmul(tP[:], ones_row[0:1, :], temp_sb[:],
                                     start=True, stop=True)
                    nc.scalar.copy(temp_col[:], tP[:])
                    nc.scalar.activation(negtemp_col[:], tP[:], AF.Copy,
                                         scale=-1.0)

                    # ---- phase 1: 1x1 conv (qkv_w), q/k col-packed, v separate
                    for n in range(8):
                        s = slice(512 * n, 512 * (n + 1))
                        T1 = psA.tile([128, 512], F32, tag="a")
                        nc.tensor.matmul(T1[0:48, :], w1a[:, 0:48], x_a[:, s],
                                         start=True, stop=False)
                        nc.tensor.matmul(T1[64:112, :], w1a[:, 48:96], x_a[:, s],
                                         start=True, stop=False)
                        nc.tensor.matmul(T1[0:48, :], w1b[:, 0:48], x_b[:, s],
                                         start=False, stop=True)
                        nc.tensor.matmul(T1[64:112, :], w1b[:, 48:96], x_b[:, s],
                                         start=False, stop=True)
                        T1v = psB.tile([48, 512], F32, tag="b")
                        nc.tensor.matmul(T1v[:], w1a[:, 96:144], x_a[:, s],
                                         start=True, stop=False)
                        nc.tensor.matmul(T1v[:], w1b[:, 96:144], x_b[:, s],
                                         start=False, stop=True)
                        if n % 2 == 0:
                            nc.vector.tensor_copy(TQK[:, s], T1[:])
                            nc.scalar.copy(TV[:, s], T1v[:])
                        else:
                            nc.scalar.copy(TQK[:, s], T1[:])
                            nc.vector.tensor_copy(TV[:, s], T1v[:])

                ph35 = tc.tile_pool(name="ph35", bufs=1)
                p35 = ph35.__enter__()
                SQ = p35.tile([112, HW], F32, tag="SQ")
                ss_sb = p35.tile([33, HW], F32, tag="ss_sb")
                ssqC = p35.tile([128, 32], F32, tag="ssqC")
                sskC = p35.tile([128, 32], F32, tag="sskC")
                lnb = p35.tile([128, 64], F32, tag="lnb")

                # ---- phase 2: depthwise 3x3 (block-diag qk lhsT, 9 taps)
                TQK3 = TQK[:].rearrange("p (y x) -> p y x", x=64)
                TV3 = TV[:].rearrange("p (y x) -> p y x", x=64)
                taps = [(0, 0)] + [(dy, dx) for dy in (-1, 0, 1)
                                   for dx in (-1, 0, 1) if (dy, dx) != (0, 0)]
                for n in range(8):
                    s = slice(512 * n, 512 * (n + 1))
                    DQK = psA.tile([128, 8, 64], F32, tag="a")
                    DV = psB.tile([48, 8, 64], F32, tag="b")
                    for ti, (dy, dx) in enumerate(taps):
                        t = (dy + 1) * 3 + (dx + 1)
                        first = ti == 0
                        last = ti == len(taps) - 1
                        gy0, gy1 = max(0, -dy), 64 - max(0, dy)
                        sy0, sy1 = max(8 * n, gy0), min(8 * n + 8, gy1)
                        if sy1 <= sy0:
                            continue
                        x0, x1 = max(0, -dx), 64 - max(0, dx)
                        oy = slice(sy0 - 8 * n, sy1 - 8 * n)
                        ox = slice(x0, x1)
                        iy = slice(sy0 + dy, sy1 + dy)
                        ix = slice(x0 + dx, x1 + dx)
                        wsl = slice(C * t, C * t + 48)
                        nc.tensor.matmul(
                            DQK[0:48, oy, ox], dwqk[0:48, wsl],
                            TQK3[0:48, iy, ix], start=first, stop=last,
                            skip_group_check=True)
                        nc.tensor.matmul(
                            DQK[64:112, oy, ox], dwqk[64:112, wsl],
                            TQK3[64:112, iy, ix], start=first, stop=last,
                            skip_group_check=True)
                        nc.tensor.matmul(
                            DV[:, oy, ox], dwv[:, wsl],
                            TV3[:, iy, ix], start=first, stop=last,
                            skip_group_check=True)
                    DQKf = DQK[:].rearrange("p y x -> p (y x)")
                    DVf = DV[:].rearrange("p y x -> p (y x)")
                    if n % 2 == 0:
                        nc.vector.tensor_copy(PK[:, s], DQKf[:])
                        nc.scalar.copy(v_sb[:, s], DVf[:])
                    else:
                        nc.scalar.copy(PK[:, s], DQKf[:])
                        nc.vector.tensor_copy(v_sb[:, s], DVf[:])

                # ---- phase 3: squares + sum-of-squares (per position);
                # partial reshape DMAs [1,512] row -> [16,32] column slab
                for n in range(8):
                    s = slice(512 * n, 512 * (n + 1))
                    ps = slice(16 * n, 16 * (n + 1))
                    nc.vector.tensor_mul(SQ[:, s], PK[0:112, s], PK[0:112, s])
                    ssP = psA.tile([128, 512], F32, tag="a")
                    nc.tensor.matmul(ssP[0:1, :], ones48[0:48, :], SQ[0:48, s],
                                     start=True, stop=True)
                    nc.tensor.matmul(ssP[32:33, :], ones48[64:112, :],
                                     SQ[64:112, s], start=True, stop=True)
                    nc.vector.tensor_copy(ss_sb[:, s], ssP[0:33, :])
                    nc.sync.dma_start(ssqC[ps, :], ss_sb[0:1, s])
                    nc.sync.dma_start(sskC[ps, :], ss_sb[32:33, s])

                # ---- phase 4: rsqrt via exp(-0.5*ln(ss)), flatten to rows
                nc.scalar.activation(lnb[:, 0:32], ssqC[:], AF.Ln)
                nc.scalar.activation(lnb[:, 32:64], sskC[:], AF.Ln)
                nc.scalar.activation(rr2[:], lnb[:], AF.Exp, scale=-0.5)
                nc.scalar.activation(rkt[:], rr2[:, 32:64], AF.Copy,
                                     scale=temp_col[:])
                nc.sync.dma_start(rq_row[0:1, :], rr2[:, 0:32])
                nc.sync.dma_start(rkt_row[0:1, :], rkt[:])
                ph35.__exit__(None, None, None)

            # ---- phase 7: attention, 8 q-blocks of 512. Block 0 interleaves
            # the remaining setup (q/k scaling + v transposes) via prep_n so
            # the PE never drains between setup and attention.
            with (
                tc.tile_pool(name="psS", bufs=2, space="PSUM") as psS,
                tc.tile_pool(name="psAV", bufs=1, space="PSUM") as psAV,
                tc.tile_pool(name="psZ", bufs=1, space="PSUM") as psZ,
                tc.tile_pool(name="zp", bufs=2) as zp,
                tc.tile_pool(name="sqp", bufs=2) as sqp,
            ):
                ntile = (KCH + UPT - 1) // UPT  # 11 tiles per block

                def prep_n(n):
                    # finish q-hat / k-tilde for column block n (PSUM-free:
                    # gpsimd broadcast + DVE mul + DMA dup) and transpose the
                    # four v chunks of block n on the scalar DMA queue
                    s = slice(512 * n, 512 * (n + 1))
                    rqb = sqp.tile([48, 512], F32, tag="rqb")
                    nc.gpsimd.partition_broadcast(rqb[:], rq_row[0:1, s])
                    nc.vector.tensor_mul(QHD[0:48, s], PK[0:48, s], rqb[:])
                    nc.sync.dma_start(QHD[64:112, s], QHD[0:48, s])
                    rkb = sqp.tile([112, 512], F32, tag="rkb")
                    nc.gpsimd.partition_broadcast(rkb[:], rkt_row[0:1, s])
                    nc.vector.tensor_mul(KHb[64:112, s], PK[64:112, s],
                                         rkb[64:112, :])
                    nc.sync.dma_start(KHb[0:48, s], KHb[64:112, s])
                    for c in range(4 * n, 4 * n + 4):
                        nc.scalar.dma_start(vT[:, 64 * c:64 * c + 48],
                                            v_sb[:, 128 * c:128 * (c + 1)],
                                            transpose=True)

                def zchain(g, step):
                    # deferred z-chain for block g, emitted piecewise.
                    # reshape DMAs: [1,512] <-> [128,4] round trip preserves
                    # element order, so no transposes needed around recip.
                    blk = slice(512 * g, 512 * (g + 1))
                    if step == 0:
                        zc = zp.tile([128, 4], F32, tag="zc")
                        nc.sync.dma_start(zc[:], U[48:49, blk])
                        zchain.zc = zc
                    elif step == 1:
                        nc.vector.reciprocal(rz[:, 4 * g:4 * g + 4],
                                             zchain.zc[:])
                    elif step == 2:
                        nc.sync.dma_start(rz_row[0:1, blk],
                                          rz[:, 4 * g:4 * g + 4])
                    elif step == 3:
                        rbP = psZ.tile([48, 512], F32, tag="z")
                        nc.tensor.matmul(rbP[:], ones_row[0:1, 0:48],
                                         rz_row[0:1, blk], start=True,
                                         stop=True)
                        zchain.rbP = rbP
                    elif step == 4:
                        nc.vector.tensor_mul(att[0:48, blk], U[0:48, blk],
                                             zchain.rbP[:])
                    elif step == 5:
                        nc.sync.dma_start(att[64:112, blk], att[0:48, blk])

                def emit_av(avP, rec):
                    cc0, cnu, cE = rec
                    for p in range(cnu):
                        c = cc0 + p
                        nc.tensor.matmul(
                            avP[:], vT[:, 64 * c:64 * c + 49],
                            cE[:, 512 * p:512 * (p + 1)],
                            start=(c == 0), stop=(c == KCH - 1),
                            skip_group_check=True)

                pend = []  # (c0, nu, E) tiles of the CURRENT block
                avP = None
                prep_done = 0
                for g in range(NB):
                    blk = slice(512 * g, 512 * (g + 1))
                    for j in range(ntile):
                        c0 = UPT * j
                        nu = min(UPT, KCH - c0)
                        if g == 0:
                            # one block ahead of the chunks tile j consumes
                            want = min(NB, (3 * j + 2) // 4 + 2)
                            while prep_done < want:
                                prep_n(prep_done)
                                prep_done += 1
                        T = psS.tile([128, UPT * 512], F32, tag="S")
                        for p in range(nu):
                            c = c0 + p
                            base = 0 if c % 2 == 0 else 64
                            nc.tensor.matmul(
                                T[:, 512 * p:512 * (p + 1)],
                                KHb[base:base + 48, 128 * c:128 * (c + 1)],
                                QHD[base:base + 48, blk],
                                start=True, stop=True)
                        E = ep.tile([128, UPT * 512], BF16, tag="E")
                        nc.scalar.activation(E[:, 0:512 * nu], T[:, 0:512 * nu],
                                             AF.Exp, bias=negtemp_col[:])
                        if j == 0 and g > 0:
                            # previous block's AV tail + U copy, after this
                            # block's first S tile keeps the PE streaming
                            while pend:
                                emit_av(avP, pend.pop(0))
                            nc.vector.tensor_copy(U[:, 512 * (g - 1):512 * g],
                                                  avP[:])
                        pend.append((c0, nu, E))
                        if j == 1:
                            avP = psAV.tile([49, 512], F32, tag="av")
                        if j >= 2:
                            emit_av(avP, pend.pop(0))
                        if g > 0 and 2 <= j <= 7:
                            zchain(g - 1, j - 2)
                while pend:
                    emit_av(avP, pend.pop(0))
                nc.vector.tensor_copy(U[:, 512 * (NB - 1):], avP[:])
                for step in range(6):
                    zchain(NB - 1, step)

            if debug_taps:
                nc.sync.dma_start(dbg_khb[:], KHb[:])
                nc.sync.dma_start(dbg_qhd[:], QHD[:])
                nc.sync.dma_start(dbg_vt[:], vT[:])
                nc.sync.dma_start(dbg_u[:], U[:])
                nc.sync.dma_start(dbg_att[:], att[:])

            # ---- phase 8: proj, DMA out straight from PSUM
            with (
                tc.tile_pool(name="ph8", bufs=1) as p8,
                tc.tile_pool(name="psE", bufs=4, space="PSUM") as psE,
                tc.tile_pool(name="psF", bufs=2, space="PSUM") as psF,
            ):
                out_sb = p8.tile([128, HW], F32, tag="out_sb")
                out_sb2 = p8.tile([64, HW], F32, tag="out_sb2")
                for n in range(8):
                    s = slice(512 * n, 512 * (n + 1))
                    oP = psE.tile([128, 512], F32, tag="e")
                    oP2 = psF.tile([64, 512], F32, tag="f")
                    nc.tensor.matmul(oP[:], pw[0:48, 0:128], att[0:48, s],
                                     start=True, stop=True)
                    nc.tensor.matmul(oP2[:], pw[64:112, 0:64], att[64:112, s],
                                     start=True, stop=True)
                    if n % 2 == 0:
                        nc.vector.tensor_copy(out_sb[:, s], oP[:])
                        nc.scalar.copy(out_sb2[:, s], oP2[:])
                    else:
                        nc.scalar.copy(out_sb[:, s], oP[:])
                        nc.vector.tensor_copy(out_sb2[:, s], oP2[:])
                    nc.sync.dma_start(out_d[0:128, s], out_sb[:, s])
                    nc.sync.dma_start(out_d[128:192, s], out_sb2[:, s])

    nc.compile()
    return nc


def _get_nc():
    if "nc" not in _cache:
        _cache["nc"] = _build()
    return _cache["nc"]


def _prep_core(x, qkv_w, dw_w, proj_w, temperature, b, h):
    w1 = qkv_w[:, :, 0, 0]  # [576, 192]
    dw = dw_w[:, 0]  # [576, 3, 3]
    pwf = proj_w[:, :, 0, 0]  # [192, 192]
    qs, ks, vs = h * C, DIM + h * C, 2 * DIM + h * C
    sel = np.concatenate(
        [w1[qs:qs + C], w1[ks:ks + C], w1[vs:vs + C]], 0)  # [144, 192]
    lhsT = np.ascontiguousarray(sel.T)  # [192, 144]
    dq, dk, dv = dw[qs:qs + C], dw[ks:ks + C], dw[vs:vs + C]
    dwqk = np.zeros((128, 9, C), np.float32)
    dwv = np.zeros((C, 9, C), np.float32)
    ar = np.arange(C)
    for t in range(9):
        dy, dx = t // 3 - 1, t % 3 - 1
        dwqk[ar, t, ar] = dq[:, dy + 1, dx + 1]
        dwqk[64 + ar, t, ar] = dk[:, dy + 1, dx + 1]
        dwv[ar, t, ar] = dv[:, dy + 1, dx + 1]
    pw_sel = pwf[:, h * C:(h + 1) * C].T  # [48, 192]
    pwt = np.zeros((128, 128), np.float32)
    pwt[0:48, 0:128] = pw_sel[:, 0:128]
    pwt[64:112, 0:64] = pw_sel[:, 128:192]
    return {
        "x": np.ascontiguousarray(x[b].reshape(DIM, HW)).astype(np.float16),
        "w1a": lhsT[0:128].astype(np.float16),
        "w1b": lhsT[128:192].astype(np.float16),
        "dwqk": dwqk.reshape(128, 9 * C).astype(np.float16),
        "dwv": dwv.reshape(C, 9 * C).astype(np.float16),
        "pw": pwt.astype(np.float16),
        "ident": np.eye(128, dtype=np.float32),
        "temp": np.array([[temperature[h, 0, 0]]], np.float32),
    }


def kernel(x, qkv_w, dw_w, proj_w, temperature):
    from concourse.bass_utils import run_bass_kernel_spmd

    nc = _get_nc()
    x = np.asarray(x, np.float32)
    qkv_w = np.asarray(qkv_w, np.float32)
    dw_w = np.asarray(dw_w, np.float32)
    proj_w = np.asarray(proj_w, np.float32)
    temperature = np.asarray(temperature, np.float32)
    in_maps = [
        _prep_core(x, qkv_w, dw_w, proj_w, temperature, c // HEADS, c % HEADS)
        for c in range(NCORES)
    ]
    res = run_bass_kernel_spmd(nc, in_maps, core_ids=list(range(NCORES)))
    out = np.zeros((B, DIM, HW), np.float32)
    for c in range(NCORES):
        out[c // HEADS] += res.results[c]["out"]
    return out.reshape(B, DIM, H, W)


# revision 39
# speedup vs baseline: 1.2077x; 1.0190x over previous
import sys

sys.path.insert(0, "/opt/trn_rl_repo")
import numpy as np

B, DIM, H, W = 2, 192, 64, 64
HEADS = 4
C = DIM // HEADS  # 48 per-head channels
HW = H * W  # 4096
NCORES = 8
KCH = HW // 128  # 32 k-chunks
NB = 8  # q blocks of 512
QW = HW // NB  # 512
UPT = 3  # S-chunk units per exp tile

_cache = {}


def _build(debug_taps=False):
    import concourse.bass as bass
    import concourse.tile as tile
    from concourse import bacc, mybir

    F32 = mybir.dt.float32
    BF16 = mybir.dt.float16
    AF = mybir.ActivationFunctionType

    nc = bacc.Bacc("TRN2", target_bir_lowering=False, debug=False,
                   num_devices=NCORES)
    x_d = nc.dram_tensor("x", [DIM, HW], BF16, kind="ExternalInput").ap()
    w1a_d = nc.dram_tensor("w1a", [128, 3 * C], BF16, kind="ExternalInput").ap()
    w1b_d = nc.dram_tensor("w1b", [64, 3 * C], BF16, kind="ExternalInput").ap()
    dwqk_d = nc.dram_tensor("dwqk", [128, 9 * C], BF16, kind="ExternalInput").ap()
    dwv_d = nc.dram_tensor("dwv", [C, 9 * C], BF16, kind="ExternalInput").ap()
    pw_d = nc.dram_tensor("pw", [128, 128], BF16, kind="ExternalInput").ap()
    id_d = nc.dram_tensor("ident", [128, 128], F32, kind="ExternalInput").ap()
    tp_d = nc.dram_tensor("temp", [1, 1], F32, kind="ExternalInput").ap()
    out_d = nc.dram_tensor("out", [DIM, HW], F32, kind="ExternalOutput").ap()
    if debug_taps:
        dbg_khb = nc.dram_tensor("dbg_khb", [128, HW], BF16,
                                 kind="ExternalOutput").ap()
        dbg_qhd = nc.dram_tensor("dbg_qhd", [128, HW], BF16,
                                 kind="ExternalOutput").ap()
        dbg_vt = nc.dram_tensor("dbg_vt", [128, KCH * 64], BF16,
                                kind="ExternalOutput").ap()
        dbg_u = nc.dram_tensor("dbg_u", [49, HW], F32,
                               kind="ExternalOutput").ap()
        dbg_att = nc.dram_tensor("dbg_att", [128, HW], BF16,
                                 kind="ExternalOutput").ap()

    with tile.TileContext(nc) as tc:
        with (
            tc.tile_pool(name="persist", bufs=1) as pp,
            tc.tile_pool(name="epool", bufs=4) as ep,
        ):
            # ---- persistent SBUF tiles
            PK = pp.tile([128, HW], F32, tag="PK")     # q'@0:48, k'@64:112
            QHD = pp.tile([128, HW], BF16, tag="QHD")  # qhat bf16 dup 0:48/64:112
            KHb = pp.tile([128, HW], BF16, tag="KHb")  # temp*rk*k' dup'd
            # v^T chunks at 64-col stride (XBAR dest needs 128B alignment);
            # col 64c+48 is the ones column for the Z row
            vT = pp.tile([128, KCH * 64], BF16, tag="vT")
            v_sb = pp.tile([C, HW], BF16, tag="v_sb")
            U = pp.tile([49, HW], F32, tag="U")        # av accum (row 48 = Z)
            rq_row = pp.tile([1, HW], F32, tag="rq_row")
            rkt_row = pp.tile([1, HW], F32, tag="rkt_row")
            ident = pp.tile([128, 128], F32, tag="ident")
            w1a = pp.tile([128, 3 * C], BF16, tag="w1a")
            w1b = pp.tile([64, 3 * C], BF16, tag="w1b")
            dwqk = pp.tile([128, 9 * C], BF16, tag="dwqk")
            dwv = pp.tile([C, 9 * C], BF16, tag="dwv")
            pw = pp.tile([128, 128], BF16, tag="pw")
            temp_sb = pp.tile([1, 1], F32, tag="temp")
            ones_row = pp.tile([1, 128], F32, tag="ones_row")
            ones48 = pp.tile([128, 1], F32, tag="ones48")
            temp_col = pp.tile([128, 1], F32, tag="temp_col")
            negtemp_col = pp.tile([128, 1], F32, tag="negtemp_col")
            rr2 = pp.tile([128, 64], F32, tag="rr2")   # rq | rk (1/||.||)
            rkt = pp.tile([128, KCH], F32, tag="rkt")  # temp * rk, [p, chunk]
            att = pp.tile([128, HW], BF16, tag="att")
            rz_row = pp.tile([1, HW], F32, tag="rz_row")
            rz = pp.tile([128, KCH], F32, tag="rz")

            nc.sync.dma_start(ident[:], id_d[:])
            nc.sync.dma_start(w1a[:], w1a_d[:])
            nc.sync.dma_start(w1b[:], w1b_d[:])
            nc.sync.dma_start(dwqk[:], dwqk_d[:])
            nc.sync.dma_start(dwv[:], dwv_d[:])
            nc.sync.dma_start(pw[:], pw_d[:])
            nc.sync.dma_start(temp_sb[:], tp_d[:])
            nc.gpsimd.memset(ones_row[:], 1.0)
            nc.gpsimd.memset(ones48[:], 1.0)
            nc.gpsimd.memset(vT[:], 1.0)


            with (
                tc.tile_pool(name="ph12", bufs=1) as p12,
                tc.tile_pool(name="psA", bufs=4, space="PSUM") as psA,
                tc.tile_pool(name="psB", bufs=2, space="PSUM") as psB,
            ):
                TQK = p12.tile([128, HW], BF16, tag="TQK")
                TV = p12.tile([C, HW], BF16, tag="TV")

                with tc.tile_pool(name="xp", bufs=1) as xp:
                    x_a = xp.tile([128, HW], BF16, tag="x_a")
                    x_b = xp.tile([64, HW], BF16, tag="x_b")
                    nc.sync.dma_start(x_a[:], x_d[0:128, :])
                    nc.sync.dma_start(x_b[:], x_d[128:192, :])

                    # temp_col = broadcast temp over partitions (K=1 matmul)
                    tP = psB.tile([128, 1], F32, tag="b")
                    nc.tensor.matmul(tP[:], ones_row[0:1, :], temp_sb[:],
                                     start=True, stop=True)
                    nc.scalar.copy(temp_col[:], tP[:])
                    nc.scalar.activation(negtemp_col[:], tP[:], AF.Copy,
                                         scale=-1.0)

                    # ---- phase 1: 1x1 conv (qkv_w), q/k col-packed, v separate
                    for n in range(8):
                        s = slice(512 * n, 512 * (n + 1))
                        T1 = psA.tile([128, 512], F32, tag="a")
                        nc.tensor.matmul(T1[0:48, :], w1a[:, 0:48], x_a[:, s],
                                         start=True, stop=False)
                        nc.tensor.matmul(T1[64:112, :], w1a[:, 48:96], x_a[:, s],
                                         start=True, stop=False)
                        nc.tensor.matmul(T1[0:48, :], w1b[:, 0:48], x_b[:, s],
                                         start=False, stop=True)
                        nc.tensor.matmul(T1[64:112, :], w1b[:, 48:96], x_b[:, s],
                                         start=False, stop=True)
                        T1v = psB.tile([48, 512], F32, tag="b")
                        nc.tensor.matmul(T1v[:], w1a[:, 96:144], x_a[:, s],
                                         start=True, stop=False)
                        nc.tensor.matmul(T1v[:], w1b[:, 96:144], x_b[:, s],
                                         start=False, stop=True)
                        if n % 2 == 0:
                            nc.vector.tensor_copy(TQK[:, s], T1[:])
                            nc.scalar.copy(TV[:, s], T1v[:])
                        else:
                            nc.scalar.copy(TQK[:, s], T1[:])
                            nc.vector.tensor_copy(TV[:, s], T1v[:])

                ph35 = tc.tile_pool(name="ph35", bufs=1)
                p35 = ph35.__enter__()
                SQ = p35.tile([112, HW], F32, tag="SQ")
                ss_sb = p35.tile([33, HW], F32, tag="ss_sb")
                ssqC = p35.tile([128, 32], F32, tag="ssqC")
                sskC = p35.tile([128, 32], F32, tag="sskC")
                lnb = p35.tile([128, 64], F32, tag="lnb")

                # ---- phase 2: depthwise 3x3 (block-diag qk lhsT, 9 taps)
                TQK3 = TQK[:].rearrange("p (y x) -> p y x", x=64)
                TV3 = TV[:].rearrange("p (y x) -> p y x", x=64)
                taps = [(0, 0)] + [(dy, dx) for dy in (-1, 0, 1)
                                   for dx in (-1, 0, 1) if (dy, dx) != (0, 0)]
                for n in range(8):
                    s = slice(512 * n, 512 * (n + 1))
                    DQK = psA.tile([128, 8, 64], F32, tag="a")
                    DV = psB.tile([48, 8, 64], F32, tag="b")
                    for ti, (dy, dx) in enumerate(taps):
                        t = (dy + 1) * 3 + (dx + 1)
                        first = ti == 0
                        last = ti == len(taps) - 1
                        gy0, gy1 = max(0, -dy), 64 - max(0, dy)
                        sy0, sy1 = max(8 * n, gy0), min(8 * n + 8, gy1)
                        if sy1 <= sy0:
                            continue
                        x0, x1 = max(0, -dx), 64 - max(0, dx)
                        oy = slice(sy0 - 8 * n, sy1 - 8 * n)
                        ox = slice(x0, x1)
                        iy = slice(sy0 + dy, sy1 + dy)
                        ix = slice(x0 + dx, x1 + dx)
                        wsl = slice(C * t, C * t + 48)
                        nc.tensor.matmul(
                            DQK[0:48, oy, ox], dwqk[0:48, wsl],
                            TQK3[0:48, iy, ix], start=first, stop=last,
                            skip_group_check=True)
                        nc.tensor.matmul(
                            DQK[64:112, oy, ox], dwqk[64:112, wsl],
                            TQK3[64:112, iy, ix], start=first, stop=last,
                            skip_group_check=True)
                        nc.tensor.matmul(
                            DV[:, oy, ox], dwv[:, wsl],
                            TV3[:, iy, ix], start=first, stop=last,
                            skip_group_check=True)
                    DQKf = DQK[:].rearrange("p y x -> p (y x)")
                    DVf = DV[:].rearrange("p y x -> p (y x)")
                    if n % 2 == 0:
                        nc.vector.tensor_copy(PK[:, s], DQKf[:])
                        nc.scalar.copy(v_sb[:, s], DVf[:])
                    else:
                        nc.scalar.copy(PK[:, s], DQKf[:])
                        nc.vector.tensor_copy(v_sb[:, s], DVf[:])

                # ---- phase 3: squares + sum-of-squares (per position);
                # partial reshape DMAs [1,512] row -> [16,32] column slab
                for n in range(8):
                    s = slice(512 * n, 512 * (n + 1))
                    ps = slice(16 * n, 16 * (n + 1))
                    nc.vector.tensor_mul(SQ[:, s], PK[0:112, s], PK[0:112, s])
                    ssP = psA.tile([128, 512], F32, tag="a")
                    nc.tensor.matmul(ssP[0:1, :], ones48[0:48, :], SQ[0:48, s],
                                     start=True, stop=True)
                    nc.tensor.matmul(ssP[32:33, :], ones48[64:112, :],
                                     SQ[64:112, s], start=True, stop=True)
                    nc.vector.tensor_copy(ss_sb[:, s], ssP[0:33, :])
                    nc.sync.dma_start(ssqC[ps, :], ss_sb[0:1, s])
                    nc.sync.dma_start(sskC[ps, :], ss_sb[32:33, s])

                # ---- phase 4: rsqrt via exp(-0.5*ln(ss)), flatten to rows
                nc.scalar.activation(lnb[:, 0:32], ssqC[:], AF.Ln)
                nc.scalar.activation(lnb[:, 32:64], sskC[:], AF.Ln)
                nc.scalar.activation(rr2[:], lnb[:], AF.Exp, scale=-0.5)
                nc.scalar.activation(rkt[:], rr2[:, 32:64], AF.Copy,
                                     scale=temp_col[:])
                nc.sync.dma_start(rq_row[0:1, :], rr2[:, 0:32])
                nc.sync.dma_start(rkt_row[0:1, :], rkt[:])
                ph35.__exit__(None, None, None)

            # ---- phase 7: attention, 8 q-blocks of 512. Block 0 interleaves
            # the remaining setup (q/k scaling + v transposes) via prep_n so
            # the PE never drains between setup and attention.
            with (
                tc.tile_pool(name="psS", bufs=2, space="PSUM") as psS,
                tc.tile_pool(name="psAV", bufs=1, space="PSUM") as psAV,
                tc.tile_pool(name="psZ", bufs=1, space="PSUM") as psZ,
                tc.tile_pool(name="zp", bufs=2) as zp,
                tc.tile_pool(name="sqp", bufs=2) as sqp,
            ):
                ntile = (KCH + UPT - 1) // UPT  # 11 tiles per block

                def prep_n(n):
                    # finish q-hat / k-tilde for column block n. Broadcasts
                    # are K=1 matmuls through the psZ bank (free during
                    # block 0 — z-chains start at block 1), keeping the PE
                    # stream dense; v transposes ride the scalar DMA queue.
                    s = slice(512 * n, 512 * (n + 1))
                    rqbP = psZ.tile([48, 512], F32, tag="z")
                    nc.tensor.matmul(rqbP[:], ones_row[0:1, 0:48],
                                     rq_row[0:1, s], start=True, stop=True)
                    nc.vector.tensor_mul(QHD[0:48, s], PK[0:48, s], rqbP[:])
                    nc.sync.dma_start(QHD[64:112, s], QHD[0:48, s])
                    rkbP = psZ.tile([128, 512], F32, tag="z")
                    nc.tensor.matmul(rkbP[64:112, :], ones_row[0:1, 0:48],
                                     rkt_row[0:1, s], start=True, stop=True)
                    nc.vector.tensor_mul(KHb[64:112, s], PK[64:112, s],
                                         rkbP[64:112, :])
                    nc.sync.dma_start(KHb[0:48, s], KHb[64:112, s])
                    for c in range(4 * n, 4 * n + 4):
                        nc.scalar.dma_start(vT[:, 64 * c:64 * c + 48],
                                            v_sb[:, 128 * c:128 * (c + 1)],
                                            transpose=True)

                def zchain(g, step):
                    # deferred z-chain for block g, emitted piecewise.
                    # reshape DMAs: [1,512] <-> [128,4] round trip preserves
                    # element order, so no transposes needed around recip.
                    blk = slice(512 * g, 512 * (g + 1))
                    if step == 0:
                        zc = zp.tile([128, 4], F32, tag="zc")
                        nc.sync.dma_start(zc[:], U[48:49, blk])
                        zchain.zc = zc
                    elif step == 1:
                        nc.vector.reciprocal(rz[:, 4 * g:4 * g + 4],
                                             zchain.zc[:])
                    elif step == 2:
                        nc.sync.dma_start(rz_row[0:1, blk],
                                          rz[:, 4 * g:4 * g + 4])
                    elif step == 3:
                        rbP = psZ.tile([48, 512], F32, tag="z")
                        nc.tensor.matmul(rbP[:], ones_row[0:1, 0:48],
                                         rz_row[0:1, blk], start=True,
                                         stop=True)
                        zchain.rbP = rbP
                    elif step == 4:
                        nc.vector.tensor_mul(att[0:48, blk], U[0:48, blk],
                                             zchain.rbP[:])
                    elif step == 5:
                        nc.sync.dma_start(att[64:112, blk], att[0:48, blk])

                def emit_av(avP, rec):
                    cc0, cnu, cE = rec
                    for p in range(cnu):
                        c = cc0 + p
                        nc.tensor.matmul(
                            avP[:], vT[:, 64 * c:64 * c + 49],
                            cE[:, 512 * p:512 * (p + 1)],
                            start=(c == 0), stop=(c == KCH - 1),
                            skip_group_check=True)

                pend = []  # (c0, nu, E) tiles of the CURRENT block
                avP = None
                prep_done = 0
                for g in range(NB):
                    blk = slice(512 * g, 512 * (g + 1))
                    for j in range(ntile):
                        c0 = UPT * j
                        nu = min(UPT, KCH - c0)
                        if g == 0:
                            # one block ahead of the chunks tile j consumes
                            want = min(NB, (3 * j + 2) // 4 + 2)
                            while prep_done < want:
                                prep_n(prep_done)
                                prep_done += 1
                        T = psS.tile([128, UPT * 512], F32, tag="S")
                        for p in range(nu):
                            c = c0 + p
                            base = 0 if c % 2 == 0 else 64
                            nc.tensor.matmul(
                                T[:, 512 * p:512 * (p + 1)],
                                KHb[base:base + 48, 128 * c:128 * (c + 1)],
                                QHD[base:base + 48, blk],
                                start=True, stop=True)
                        E = ep.tile([128, UPT * 512], BF16, tag="E")
                        nc.scalar.activation(E[:, 0:512 * nu], T[:, 0:512 * nu],
                                             AF.Exp, bias=negtemp_col[:])
                        if j == 0 and g > 0:
                            # previous block's AV tail + U copy, after this
                            # block's first S tile keeps the PE streaming
                            while pend:
                                emit_av(avP, pend.pop(0))
                            nc.vector.tensor_copy(U[:, 512 * (g - 1):512 * g],
                                                  avP[:])
                        pend.append((c0, nu, E))
                        if j == 1:
                            avP = psAV.tile([49, 512], F32, tag="av")
                        if j >= 2:
                            emit_av(avP, pend.pop(0))
                        if g > 0 and 2 <= j <= 7:
                            zchain(g - 1, j - 2)
                while pend:
                    emit_av(avP, pend.pop(0))
                nc.vector.tensor_copy(U[:, 512 * (NB - 1):], avP[:])
                for step in range(6):
                    zchain(NB - 1, step)

            if debug_taps:
                nc.sync.dma_start(dbg_khb[:], KHb[:])
                nc.sync.dma_start(dbg_qhd[:], QHD[:])
                nc.sync.dma_start(dbg_vt[:], vT[:])
                nc.sync.dma_start(dbg_u[:], U[:])
                nc.sync.dma_start(dbg_att[:], att[:])

            # ---- phase 8: proj, DMA out straight from PSUM
            with (
                tc.tile_pool(name="ph8", bufs=1) as p8,
                tc.tile_pool(name="psE", bufs=4, space="PSUM") as psE,
                tc.tile_pool(name="psF", bufs=2, space="PSUM") as psF,
            ):
                out_sb = p8.tile([128, HW], F32, tag="out_sb")
                out_sb2 = p8.tile([64, HW], F32, tag="out_sb2")
                for n in range(8):
                    s = slice(512 * n, 512 * (n + 1))
                    oP = psE.tile([128, 512], F32, tag="e")
                    oP2 = psF.tile([64, 512], F32, tag="f")
                    nc.tensor.matmul(oP[:], pw[0:48, 0:128], att[0:48, s],
                                     start=True, stop=True)
                    nc.tensor.matmul(oP2[:], pw[64:112, 0:64], att[64:112, s],
                                     start=True, stop=True)
                    if n % 2 == 0:
                        nc.vector.tensor_copy(out_sb[:, s], oP[:])
                        nc.scalar.copy(out_sb2[:, s], oP2[:])
                    else:
                        nc.scalar.copy(out_sb[:, s], oP[:])
                        nc.vector.tensor_copy(out_sb2[:, s], oP2[:])
                    nc.sync.dma_start(out_d[0:128, s], out_sb[:, s])
                    nc.sync.dma_start(out_d[128:192, s], out_sb2[:, s])

    nc.compile()
    return nc


def _get_nc():
    if "nc" not in _cache:
        _cache["nc"] = _build()
    return _cache["nc"]


def _prep_core(x, qkv_w, dw_w, proj_w, temperature, b, h):
    w1 = qkv_w[:, :, 0, 0]  # [576, 192]
    dw = dw_w[:, 0]  # [576, 3, 3]
    pwf = proj_w[:, :, 0, 0]  # [192, 192]
    qs, ks, vs = h * C, DIM + h * C, 2 * DIM + h * C
    sel = np.concatenate(
        [w1[qs:qs + C], w1[ks:ks + C], w1[vs:vs + C]], 0)  # [144, 192]
    lhsT = np.ascontiguousarray(sel.T)  # [192, 144]
    dq, dk, dv = dw[qs:qs + C], dw[ks:ks + C], dw[vs:vs + C]
    dwqk = np.zeros((128, 9, C), np.float32)
    dwv = np.zeros((C, 9, C), np.float32)
    ar = np.arange(C)
    for t in range(9):
        dy, dx = t // 3 - 1, t % 3 - 1
        dwqk[ar, t, ar] = dq[:, dy + 1, dx + 1]
        dwqk[64 + ar, t, ar] = dk[:, dy + 1, dx + 1]
        dwv[ar, t, ar] = dv[:, dy + 1, dx + 1]
    pw_sel = pwf[:, h * C:(h + 1) * C].T  # [48, 192]
    pwt = np.zeros((128, 128), np.float32)
    pwt[0:48, 0:128] = pw_sel[:, 0:128]
    pwt[64:112, 0:64] = pw_sel[:, 128:192]
    return {
        "x": np.ascontiguousarray(x[b].reshape(DIM, HW)).astype(np.float16),
        "w1a": lhsT[0:128].astype(np.float16),
        "w1b": lhsT[128:192].astype(np.float16),
        "dwqk": dwqk.reshape(128, 9 * C).astype(np.float16),
        "dwv": dwv.reshape(C, 9 * C).astype(np.float16),
        "pw": pwt.astype(np.float16),
        "ident": np.eye(128, dtype=np.float32),
        "temp": np.array([[temperature[h, 0, 0]]], np.float32),
    }


def kernel(x, qkv_w, dw_w, proj_w, temperature):
    from concourse.bass_utils import run_bass_kernel_spmd

    nc = _get_nc()
    x = np.asarray(x, np.float32)
    qkv_w = np.asarray(qkv_w, np.float32)
    dw_w = np.asarray(dw_w, np.float32)
    proj_w = np.asarray(proj_w, np.float32)
    temperature = np.asarray(temperature, np.float32)
    in_maps = [
        _prep_core(x, qkv_w, dw_w, proj_w, temperature, c // HEADS, c % HEADS)
        for c in range(NCORES)
    ]
    res = run_bass_kernel_spmd(nc, in_maps, core_ids=list(range(NCORES)))
    out = np.zeros((B, DIM, HW), np.float32)
    for c in range(NCORES):
        out[c // HEADS] += res.results[c]["out"]
    return out.reshape(B, DIM, H, W)
